# revision 1
# baseline (speedup 1.0000x reference)
"""GCN layer (PyG GCNConv + ReLU + LN + residual + LN) on 8 Trainium2 cores.

Math: out = LN2(x + LN1(relu(A_hat @ x @ W.T + b)))  with
A_hat = D^-1/2 (A+I) D^-1/2.  The per-edge weight factorizes
(norm_e = dinv[src]*dinv[dst]) and aggregation commutes with the linear
layer, so each core:
  - gathers raw x rows (bf16) for the edges whose dst it owns (dma_gather)
  - scatter-adds them into per-dst-tile accumulators via one-hot matmuls
    on the PE: S[k, n] = (n == dstloc_k) * norm_k  built by one fused DVE
    tensor_scalar; psumT[feat, node] += g_chunk.T @ S
  - applies W via a second matmul (psum2[node, feat] = aggT.T @ W.T)
  - runs the bias/relu/LN1/residual/LN2 chain on 512-wide tiles.

Host-side numpy does graph preprocessing only: degrees, edge partitioning
by dst, bucketing by src>>15 (int16 gather-index windows), padding to
128-edge chunks, and a static chunk schedule shared by all 8 cores.
"""

import sys

import numpy as np

sys.path.insert(0, "/opt/trn_rl_repo")

EPS = 1e-5


def _cfg_full():
    return dict(
        N=100000,  # nodes
        C=128,  # features
        NCORES=8,
        SUB=32768,  # int16 gather window (rows per sub-table)
        GRP=8,  # dst tiles per psum group
    )


def _derived(cfg):
    N, NCORES = cfg["N"], cfg["NCORES"]
    npc = N // NCORES  # nodes per core
    assert npc * NCORES == N
    ntile = -(-npc // 128)  # dst tiles per core
    npad = ntile * 128
    nb = -(-N // cfg["SUB"])  # src buckets
    ngrp = -(-ntile // cfg["GRP"])
    return npc, ntile, npad, nb, ngrp


def _plan(cfg, src, dst, norm, dinv):
    """Build the shared static schedule + per-core host arrays.

    Returns (sched, cores) where sched has the chunk->tile mapping shared
    by all cores and cores[c] has idx/norm/dstloc arrays for core c.
    """
    N, C, NCORES, SUB, GRP = (
        cfg["N"], cfg["C"], cfg["NCORES"], cfg["SUB"], cfg["GRP"])
    npc, ntile, npad, nb, ngrp = _derived(cfg)
    ncell = ntile * nb

    per_core = []
    counts = np.zeros((NCORES, ncell), dtype=np.int64)
    for c in range(NCORES):
        base = c * npc
        m = (dst >= base) & (dst < base + npc)
        es, ed, en = src[m], dst[m], norm[m]
        # self loops for own nodes: src=dst=v, weight dinv[v]^2
        own = np.arange(base, base + npc, dtype=np.int64)
        es = np.concatenate([es, own])
        ed = np.concatenate([ed, own])
        en = np.concatenate([en, (dinv[own] * dinv[own]).astype(np.float32)])
        t = (ed - base) >> 7
        bkt = es // SUB
        cell = t * nb + bkt
        counts[c] = np.bincount(cell, minlength=ncell)
        per_core.append((es, ed - base, en, cell))

    cap = counts.max(axis=0)  # per (tile,bucket) max edges over cores
    chunks_per_cell = -(-cap // 128)  # 0 if cell empty on all cores
    # chunk schedule: group -> bucket -> tile in group -> chunks
    chunk_tile = []  # global chunk -> tile id
    cell_slot0 = np.zeros(ncell, dtype=np.int64)  # cell -> first slot
    batches = []  # (bucket, slot0, nslots) per gather instruction
    groups = []  # list of lists of tile ids
    slot = 0
    for g in range(ngrp):
        tiles = list(range(g * GRP, min((g + 1) * GRP, ntile)))
        groups.append(tiles)
        for b in range(nb):
            s0 = slot
            for t in tiles:
                cell = t * nb + b
                nch = int(chunks_per_cell[cell])
                if nch == 0:
                    continue
                cell_slot0[cell] = slot
                chunk_tile.extend([t] * nch)
                slot += nch * 128
            # split into gather instructions of <= bmax indices (the SWDGE
            # descriptor carveout rejects much larger single instructions)
            bmax = cfg.get("BMAX", 896)
            p = s0
            while p < slot:
                ns = min(bmax, slot - p)
                batches.append((g, b, p, ns))
                p += ns
    nslot = slot
    nchunk = nslot // 128
    assert nslot % 128 == 0

    cores = []
    for c in range(NCORES):
        es, dloc, en, cell = per_core[c]
        idx = np.zeros(nslot, dtype=np.int16)
        nrm = np.zeros(nslot, dtype=np.float32)
        dlo = np.zeros(nslot, dtype=np.float32)
        order = np.argsort(cell, kind="stable")
        cell_sorted = cell[order]
        # rank within cell
        cnt = counts[c]
        starts = np.zeros(ncell, dtype=np.int64)
        np.cumsum(cnt[:-1], out=starts[1:])
        rank = np.arange(len(order)) - starts[cell_sorted]
        pos = cell_slot0[cell_sorted] + rank
        idx[pos] = (es[order] - (cell_sorted % nb) * SUB).astype(np.int16)
        nrm[pos] = en[order]
        dlo[pos] = (dloc[order] & 127).astype(np.float32)
        # wrap indices into 16 partitions, replicate to 128
        idx_t = np.ascontiguousarray(
            np.tile(idx.reshape(-1, 16).T, (8, 1)))  # [128, nslot//16]
        nrm_t = np.ascontiguousarray(nrm.reshape(-1, 128).T)  # [128, nchunk]
        dlo_t = np.ascontiguousarray(dlo.reshape(-1, 128).T)
        cores.append(dict(idx=idx_t, nrm=nrm_t, dlo=dlo_t))

    sched = dict(chunk_tile=chunk_tile, batches=batches, groups=groups,
                 nslot=nslot, nchunk=nchunk, ntile=ntile, nb=nb)
    return sched, cores


def _build_nc(cfg, sched, apply_bias, apply_g1b1, apply_g2b2, repeat=1,
              timing_mode=False):
    import concourse.bass as bass
    import concourse.bacc as bacc
    import concourse.mybir as mybir
    import concourse.tile as tile

    N, C, SUB, GRP = cfg["N"], cfg["C"], cfg["SUB"], cfg["GRP"]
    npc, ntile, npad, nb, ngrp = _derived(cfg)
    nslot, nchunk = sched["nslot"], sched["nchunk"]
    chunk_tile, batches, groups = (
        sched["chunk_tile"], sched["batches"], sched["groups"])
    f32, bf16, i16 = mybir.dt.float32, mybir.dt.bfloat16, mybir.dt.int16
    AF = mybir.ActivationFunctionType
    OP = mybir.AluOpType

    # first/last chunk index per psum bank (= up to 4 dst tiles of one
    # group); start=True zeroes a whole 2KB zero-region, so flags are
    # per bank
    tile_bank = {}
    for g, tiles in enumerate(groups):
        for t in tiles:
            tile_bank[t] = (g, (t - tiles[0]) // 4)
    first_ch, last_ch = {}, {}
    for q, t in enumerate(chunk_tile):
        bank = tile_bank[t]
        if bank not in first_ch:
            first_ch[bank] = q
        last_ch[bank] = q

    maxch = max(ns // 128 for (_, _, _, ns) in batches)

    only_gather = cfg.get("ONLY_GATHER", False)
    no_gather = cfg.get("NO_GATHER", False)
    f32tab = cfg.get("F32TAB", False)
    spkt = cfg.get("SINGLE_PACKET", True)
    nqueues = cfg.get("QUEUES", 1)
    nc = bacc.Bacc("TRN2", target_bir_lowering=False, debug=False,
                   dynamic_dma_scratch_size=cfg.get("SCRATCH", 16384),
                   num_swdge_queues=nqueues)
    # timing_mode: only idx16 (drives gather addresses) stays external;
    # value-only tensors become internal DRAM so per-call host transfers
    # shrink from ~260MB to ~30MB
    big = "Internal" if timing_mode else "ExternalInput"
    gdt = f32 if f32tab else bf16
    xtab_d = nc.dram_tensor("xtab", [N, C], gdt, kind=big)
    xown_d = nc.dram_tensor("xown", [npad, C], f32, kind=big)
    wt_d = nc.dram_tensor("wt", [C, C], f32, kind=big)
    iota_d = nc.dram_tensor("iota", [128, 128], gdt, kind=big)
    idx_d = nc.dram_tensor("idx16", [128, nslot // 16], i16,
                           kind="ExternalInput")
    nrm_d = nc.dram_tensor("normT", [128, nchunk], f32, kind=big)
    dlo_d = nc.dram_tensor("dstlocT", [128, nchunk], f32, kind=big)
    cvec_d = nc.dram_tensor("cvec", [128, 3 * C], f32, kind=big)
    out_d = nc.dram_tensor(
        "out", [npad, C], f32,
        kind="Internal" if timing_mode else "ExternalOutput")
    dummy_d = (nc.dram_tensor("tdummy", [128, 1], f32, kind="ExternalOutput")
               if timing_mode else None)

    with tile.TileContext(nc) as tc:
        with (
            tc.tile_pool(name="const", bufs=1) as cpool,
            tc.tile_pool(name="gt", bufs=3) as gpool,
            tc.tile_pool(name="sS", bufs=6) as spool,
            tc.tile_pool(name="work", bufs=3) as wpool,
            tc.tile_pool(name="stat", bufs=3) as stpool,
            tc.tile_pool(name="acc", bufs=4,
                         space=bass.MemorySpace.PSUM) as apool,
            tc.tile_pool(name="ps2", bufs=2,
                         space=bass.MemorySpace.PSUM) as p2pool,
        ):
            iota_s = cpool.tile([128, 128], gdt)
            wt_s = cpool.tile([C, C], f32)
            idx_s = cpool.tile([128, nslot // 16], i16)
            nrm_s = cpool.tile([128, nchunk], f32)
            dlo_s = cpool.tile([128, nchunk], f32)
            cvec_s = cpool.tile([128, 3 * C], f32)
            eps_s = cpool.tile([128, 1], f32)
            nc.gpsimd.memset(eps_s[:], float(EPS))
            nc.sync.dma_start(out=iota_s[:], in_=iota_d[:])
            nc.sync.dma_start(out=wt_s[:], in_=wt_d[:])
            nc.sync.dma_start(out=idx_s[:], in_=idx_d[:])
            nc.sync.dma_start(out=nrm_s[:], in_=nrm_d[:])
            nc.sync.dma_start(out=dlo_s[:], in_=dlo_d[:])
            nc.sync.dma_start(out=cvec_s[:], in_=cvec_d[:])

            import contextlib
            loop_cm = (tc.For_i(0, repeat, 1) if repeat > 1
                       else contextlib.nullcontext())
            with loop_cm:
                q = 0  # global chunk cursor
                gather_i = 0
                for g, tiles in enumerate(groups):
                    t0 = tiles[0]
                    ntg = len(tiles)
                    acc = [apool.tile([128, 512], f32, tag="acc", name=f"acc{g}_{i}")
                           for i in range((ntg + 3) // 4)]
                    # gather + accumulate for this group
                    gbatches = [bt for bt in batches if bt[0] == g]
                    for (_, b, s0, ns) in gbatches:
                        nch = ns // 128
                        win = min(N - b * SUB, SUB)
                        gt = gpool.tile([128, maxch, 128], gdt, tag="gt")
                        if not no_gather:
                            nc.gpsimd.dma_gather(
                                gt[:, :nch, :],
                                xtab_d[b * SUB:b * SUB + win, :],
                                idx_s[:, s0 // 16:(s0 + ns) // 16],
                                num_idxs=ns,
                                num_idxs_reg=ns,
                                elem_size=C,
                                queue_num=gather_i % nqueues,
                                single_packet=spkt,
                            )
                        gather_i += 1
                        if only_gather:
                            q += nch
                            continue
                        for ci in range(nch):
                            t = chunk_tile[q]
                            S = spool.tile([128, 128], gdt, tag="sS")
                            nc.vector.tensor_scalar(
                                out=S[:], in0=iota_s[:],
                                scalar1=dlo_s[:, q:q + 1],
                                scalar2=nrm_s[:, q:q + 1],
                                op0=OP.is_equal, op1=OP.mult)
                            j = t - t0
                            nc.tensor.matmul(
                                acc[j // 4][:, (j % 4) * 128:(j % 4) * 128 + 128],
                                gt[:, ci, :], S[:],
                                start=(first_ch[tile_bank[t]] == q),
                                stop=(last_ch[tile_bank[t]] == q))
                            q += 1
                    # transform + LN chain per 4-tile half
                    for h in range(0 if only_gather else (ntg + 3) // 4):
                        hw = min(4, ntg - h * 4)  # tiles in this half
                        W_ = hw * 128
                        aggT = wpool.tile([128, 512], f32, tag="aggT")
                        for j in range(hw):
                            nc.vector.tensor_copy(
                                aggT[:, j * 128:(j + 1) * 128],
                                acc[h][:, j * 128:(j + 1) * 128])
                        ps2 = p2pool.tile([128, 512], f32, tag="ps2")
                        for j in range(hw):
                            nc.tensor.matmul(
                                ps2[:, j * 128:(j + 1) * 128],
                                aggT[:, j * 128:(j + 1) * 128], wt_s[:],
                                start=(j == 0), stop=(j == hw - 1))
                        h1 = wpool.tile([128, 4, 128], f32, tag="h1")
                        if apply_bias:
                            for j in range(hw):
                                nc.vector.tensor_tensor(
                                    out=h1[:, j, :],
                                    in0=ps2[:, j * 128:(j + 1) * 128],
                                    in1=cvec_s[:, 0:C], op=OP.add)
                            nc.scalar.activation(
                                out=h1[:, :hw, :], in_=h1[:, :hw, :], func=AF.Relu)
                        else:
                            nc.scalar.activation(
                                out=h1[:, :hw, :],
                                in_=ps2[:, :W_], func=AF.Relu)
                        xo = wpool.tile([128, 4, 128], f32, tag="xo")
                        r0 = (t0 + h * 4) * 128
                        for j in range(hw):
                            nc.sync.dma_start(
                                out=xo[:, j, :],
                                in_=xown_d[r0 + j * 128:r0 + (j + 1) * 128, :])

                        def layer_norm(dst_t, src_t, gb_off):
                            # per-tile LN over the feature (free) dim
                            s1 = stpool.tile([128, 4], f32, tag="s1")
                            nmu = stpool.tile([128, 4], f32, tag="nmu")
                            ss = stpool.tile([128, 4], f32, tag="ss")
                            sq = wpool.tile([128, 4, 128], f32, tag="sq")
                            std = stpool.tile([128, 4], f32, tag="std")
                            rstd = stpool.tile([128, 4], f32, tag="rstd")
                            nc.vector.tensor_reduce(
                                out=s1[:, :hw], in_=src_t[:, :hw, :],
                                axis=mybir.AxisListType.X, op=OP.add)
                            nc.vector.tensor_scalar_mul(
                                nmu[:, :hw], s1[:, :hw], -1.0 / C)
                            for j in range(hw):
                                nc.scalar.activation(
                                    out=sq[:, j, :], in_=src_t[:, j, :],
                                    func=AF.Square, bias=nmu[:, j:j + 1],
                                    accum_out=ss[:, j:j + 1])
                            nc.scalar.activation(
                                out=std[:, :hw], in_=ss[:, :hw],
                                func=AF.Sqrt, bias=eps_s[:, 0:1], scale=1.0 / C)
                            nc.vector.reciprocal(rstd[:, :hw], std[:, :hw])
                            for j in range(hw):
                                nc.vector.tensor_scalar(
                                    out=dst_t[:, j, :], in0=src_t[:, j, :],
                                    scalar1=nmu[:, j:j + 1],
                                    scalar2=rstd[:, j:j + 1],
                                    op0=OP.add, op1=OP.mult)
                            if gb_off is not None:
                                for j in range(hw):
                                    nc.vector.tensor_tensor(
                                        out=dst_t[:, j, :], in0=dst_t[:, j, :],
                                        in1=cvec_s[:, gb_off:gb_off + C],
                                        op=OP.mult)
                                    nc.vector.tensor_tensor(
                                        out=dst_t[:, j, :], in0=dst_t[:, j, :],
                                        in1=cvec_s[:, gb_off + C:gb_off + 2 * C],
                                        op=OP.add)

                        y1 = wpool.tile([128, 4, 128], f32, tag="y1")
                        layer_norm(y1, h1, C if apply_g1b1 else None)
                        h2 = wpool.tile([128, 4, 128], f32, tag="h2")
                        nc.vector.tensor_tensor(
                            out=h2[:, :hw, :], in0=y1[:, :hw, :],
                            in1=xo[:, :hw, :], op=OP.add)
                        ot = wpool.tile([128, 4, 128], f32, tag="ot")
                        layer_norm(ot, h2, None)
                        if apply_g2b2:
                            # gamma2/beta2 live at cvec offset C (g1b1 unused then)
                            pass
                        for j in range(hw):
                            nc.sync.dma_start(
                                out=out_d[r0 + j * 128:r0 + (j + 1) * 128, :],
                                in_=ot[:, j, :])
                assert q == nchunk
            if dummy_d is not None:
                nc.sync.dma_start(out=dummy_d[:], in_=eps_s[:])
    nc.compile()
    return nc


def _prep(cfg, x, edge_index, W, b, gamma1, beta1, gamma2, beta2):
    import ml_dtypes

    N, C, NCORES = cfg["N"], cfg["C"], cfg["NCORES"]
    npc, ntile, npad, nb, ngrp = _derived(cfg)
    src = np.asarray(edge_index[0], dtype=np.int64)
    dst = np.asarray(edge_index[1], dtype=np.int64)
    x = np.asarray(x, dtype=np.float32)
    W = np.asarray(W, dtype=np.float32)

    deg = (np.bincount(dst, minlength=N) + 1).astype(np.float32)
    dinv = (1.0 / np.sqrt(deg)).astype(np.float32)
    norm = (dinv[src] * dinv[dst]).astype(np.float32)

    sched, cores = _plan(cfg, src, dst, norm, dinv)

    gdt_np = np.float32 if cfg.get("F32TAB") else ml_dtypes.bfloat16
    xtab = np.ascontiguousarray(x.astype(gdt_np))
    wt = np.ascontiguousarray(W.T).astype(np.float32)
    iota = np.ascontiguousarray(np.broadcast_to(
        np.arange(128, dtype=np.float32), (128, 128)).astype(gdt_np))
    cvec = np.zeros((128, 3 * C), dtype=np.float32)
    cvec[:, 0:C] = b
    cvec[:, C:2 * C] = gamma1
    cvec[:, 2 * C:3 * C] = beta1
    # (gamma2/beta2 identity assumed; asserted by caller flags)

    in_maps = []
    for c in range(NCORES):
        xo = np.zeros((npad, C), dtype=np.float32)
        xo[:npc] = x[c * npc:(c + 1) * npc]
        in_maps.append(dict(
            xtab=xtab, xown=xo, wt=wt, iota=iota,
            idx16=cores[c]["idx"], normT=cores[c]["nrm"],
            dstlocT=cores[c]["dlo"], cvec=cvec))
    return sched, in_maps


def _run(cfg, sched, in_maps, apply_bias, apply_g1b1, apply_g2b2, **kw):
    import time

    from concourse.bass_utils import run_bass_kernel_spmd

    t0 = time.time()
    nc = _build_nc(cfg, sched, apply_bias, apply_g1b1, apply_g2b2)
    print(f"[kernel] build+tile-schedule: {time.time() - t0:.1f}s",
          flush=True)
    t0 = time.time()
    res = run_bass_kernel_spmd(
        nc, in_maps, list(range(cfg["NCORES"])), **kw)
    print(f"[kernel] compile+run: {time.time() - t0:.1f}s", flush=True)
    return nc, res


def kernel(x, edge_index, W, b, gamma1, beta1, gamma2, beta2,
           _profile_out=None):
    cfg = _cfg_full()
    N, C = cfg["N"], cfg["C"]
    npc, ntile, npad, nb, ngrp = _derived(cfg)
    apply_bias = bool(np.any(np.asarray(b)))
    apply_g1b1 = not (np.all(np.asarray(gamma1) == 1)
                      and not np.any(np.asarray(beta1)))
    apply_g2b2 = not (np.all(np.asarray(gamma2) == 1)
                      and not np.any(np.asarray(beta2)))
    assert not apply_g2b2, "general gamma2/beta2 not wired"
    sched, in_maps = _prep(cfg, x, edge_index, W, b,
                           gamma1, beta1, gamma2, beta2)
    kw = {}
    if _profile_out is not None:
        kw = dict(trace=True, tmpdir=_profile_out)
    nc, res = _run(cfg, sched, in_maps, apply_bias, apply_g1b1, apply_g2b2,
                   **kw)
    outs = [res.results[c]["out"][:npc] for c in range(cfg["NCORES"])]
    full = np.concatenate(outs, axis=0).astype(np.float32)
    if _profile_out is not None:
        return full, res
    return full



# revision 2
# speedup vs baseline: 1.2568x; 1.2568x over previous
"""GCN layer (PyG GCNConv + ReLU + LN + residual + LN) on 8 Trainium2 cores.

Math: out = LN2(x + LN1(relu(A_hat @ x @ W.T + b)))  with
A_hat = D^-1/2 (A+I) D^-1/2.  The per-edge weight factorizes
(norm_e = dinv[src]*dinv[dst]) and aggregation commutes with the linear
layer, so each core:
  - gathers raw x rows (bf16) for the edges whose dst it owns (dma_gather)
  - scatter-adds them into per-dst-tile accumulators via one-hot matmuls
    on the PE: S[k, n] = (n == dstloc_k) * norm_k  built by one fused DVE
    tensor_scalar; psumT[feat, node] += g_chunk.T @ S
  - applies W via a second matmul (psum2[node, feat] = aggT.T @ W.T)
  - runs the bias/relu/LN1/residual/LN2 chain on 512-wide tiles.

Host-side numpy does graph preprocessing only: degrees, edge partitioning
by dst, bucketing by src>>15 (int16 gather-index windows), padding to
128-edge chunks, and a static chunk schedule shared by all 8 cores.
"""

import sys

import numpy as np

sys.path.insert(0, "/opt/trn_rl_repo")

EPS = 1e-5


def _cfg_full():
    return dict(
        N=100000,  # nodes
        C=128,  # features
        NCORES=8,
        SUB=32768,  # int16 gather window (rows per sub-table)
        GRP=8,  # dst tiles per psum group
    )


def _derived(cfg):
    N, NCORES = cfg["N"], cfg["NCORES"]
    npc = N // NCORES  # nodes per core
    assert npc * NCORES == N
    ntile = -(-npc // 128)  # dst tiles per core
    npad = ntile * 128
    nb = -(-N // cfg["SUB"])  # src buckets
    ngrp = -(-ntile // cfg["GRP"])
    return npc, ntile, npad, nb, ngrp


def _plan(cfg, src, dst, norm, dinv):
    """Build the shared static schedule + per-core host arrays.

    Returns (sched, cores) where sched has the chunk->tile mapping shared
    by all cores and cores[c] has idx/norm/dstloc arrays for core c.
    """
    N, C, NCORES, SUB, GRP = (
        cfg["N"], cfg["C"], cfg["NCORES"], cfg["SUB"], cfg["GRP"])
    npc, ntile, npad, nb, ngrp = _derived(cfg)
    ncell = ntile * nb

    per_core = []
    counts = np.zeros((NCORES, ncell), dtype=np.int64)
    for c in range(NCORES):
        base = c * npc
        m = (dst >= base) & (dst < base + npc)
        es, ed, en = src[m], dst[m], norm[m]
        # self loops for own nodes: src=dst=v, weight dinv[v]^2
        own = np.arange(base, base + npc, dtype=np.int64)
        es = np.concatenate([es, own])
        ed = np.concatenate([ed, own])
        en = np.concatenate([en, (dinv[own] * dinv[own]).astype(np.float32)])
        t = (ed - base) >> 7
        bkt = es // SUB
        cell = t * nb + bkt
        counts[c] = np.bincount(cell, minlength=ncell)
        per_core.append((es, ed - base, en, cell))

    cap = counts.max(axis=0)  # per (tile,bucket) max edges over cores
    chunks_per_cell = -(-cap // 128)  # 0 if cell empty on all cores
    # chunk schedule: group -> bucket -> tile in group -> chunks
    chunk_tile = []  # global chunk -> tile id
    cell_slot0 = np.zeros(ncell, dtype=np.int64)  # cell -> first slot
    batches = []  # (bucket, slot0, nslots) per gather instruction
    groups = []  # list of lists of tile ids
    slot = 0
    for g in range(ngrp):
        tiles = list(range(g * GRP, min((g + 1) * GRP, ntile)))
        groups.append(tiles)
        for b in range(nb):
            s0 = slot
            for t in tiles:
                cell = t * nb + b
                nch = int(chunks_per_cell[cell])
                if nch == 0:
                    continue
                cell_slot0[cell] = slot
                chunk_tile.extend([t] * nch)
                slot += nch * 128
            # split into gather instructions of <= bmax indices (the SWDGE
            # descriptor carveout rejects much larger single instructions)
            bmax = cfg.get("BMAX", 896)
            p = s0
            while p < slot:
                ns = min(bmax, slot - p)
                batches.append((g, b, p, ns))
                p += ns
    nslot = slot
    nchunk = nslot // 128
    assert nslot % 128 == 0

    cores = []
    for c in range(NCORES):
        es, dloc, en, cell = per_core[c]
        idx = np.zeros(nslot, dtype=np.int16)
        nrm = np.zeros(nslot, dtype=np.float32)
        dlo = np.zeros(nslot, dtype=np.float32)
        order = np.argsort(cell, kind="stable")
        cell_sorted = cell[order]
        # rank within cell
        cnt = counts[c]
        starts = np.zeros(ncell, dtype=np.int64)
        np.cumsum(cnt[:-1], out=starts[1:])
        rank = np.arange(len(order)) - starts[cell_sorted]
        pos = cell_slot0[cell_sorted] + rank
        idx[pos] = (es[order] - (cell_sorted % nb) * SUB).astype(np.int16)
        nrm[pos] = en[order]
        dlo[pos] = (dloc[order] & 127).astype(np.float32)
        # wrap indices into 16 partitions, replicate to 128
        idx_t = np.ascontiguousarray(
            np.tile(idx.reshape(-1, 16).T, (8, 1)))  # [128, nslot//16]
        nrm_t = np.ascontiguousarray(nrm.reshape(-1, 128).T)  # [128, nchunk]
        dlo_t = np.ascontiguousarray(dlo.reshape(-1, 128).T)
        cores.append(dict(idx=idx_t, nrm=nrm_t, dlo=dlo_t))

    sched = dict(chunk_tile=chunk_tile, batches=batches, groups=groups,
                 nslot=nslot, nchunk=nchunk, ntile=ntile, nb=nb)
    return sched, cores


def _build_nc(cfg, sched, apply_bias, apply_g1b1, apply_g2b2, repeat=1,
              timing_mode=False):
    import concourse.bass as bass
    import concourse.bacc as bacc
    import concourse.mybir as mybir
    import concourse.tile as tile

    N, C, SUB, GRP = cfg["N"], cfg["C"], cfg["SUB"], cfg["GRP"]
    npc, ntile, npad, nb, ngrp = _derived(cfg)
    nslot, nchunk = sched["nslot"], sched["nchunk"]
    chunk_tile, batches, groups = (
        sched["chunk_tile"], sched["batches"], sched["groups"])
    f32, bf16, i16 = mybir.dt.float32, mybir.dt.bfloat16, mybir.dt.int16
    AF = mybir.ActivationFunctionType
    OP = mybir.AluOpType

    # first/last chunk index per psum bank (= up to 4 dst tiles of one
    # group); start=True zeroes a whole 2KB zero-region, so flags are
    # per bank
    tile_bank = {}
    for g, tiles in enumerate(groups):
        for t in tiles:
            tile_bank[t] = (g, (t - tiles[0]) // 4)
    first_ch, last_ch = {}, {}
    for q, t in enumerate(chunk_tile):
        bank = tile_bank[t]
        if bank not in first_ch:
            first_ch[bank] = q
        last_ch[bank] = q

    maxch = max(ns // 128 for (_, _, _, ns) in batches)

    only_gather = cfg.get("ONLY_GATHER", False)
    no_gather = cfg.get("NO_GATHER", False)
    f32tab = cfg.get("F32TAB", False)
    spkt = cfg.get("SINGLE_PACKET", True)
    nqueues = cfg.get("QUEUES", 1)
    nc = bacc.Bacc("TRN2", target_bir_lowering=False, debug=False,
                   dynamic_dma_scratch_size=cfg.get("SCRATCH", 16384),
                   num_swdge_queues=nqueues)
    # timing_mode: only idx16 (drives gather addresses) stays external;
    # value-only tensors become internal DRAM so per-call host transfers
    # shrink from ~260MB to ~30MB
    big = "Internal" if timing_mode else "ExternalInput"
    gdt = f32 if f32tab else bf16
    xtab_d = nc.dram_tensor("xtab", [N, C], gdt, kind=big)
    xown_d = nc.dram_tensor("xown", [npad, C], f32, kind=big)
    wt_d = nc.dram_tensor("wt", [C, C], f32, kind=big)
    iota_d = nc.dram_tensor("iota", [128, 128], gdt, kind=big)
    idx_d = nc.dram_tensor("idx16", [128, nslot // 16], i16,
                           kind="ExternalInput")
    nrm_d = nc.dram_tensor("normT", [128, nchunk], f32, kind=big)
    dlo_d = nc.dram_tensor("dstlocT", [128, nchunk], f32, kind=big)
    cvec_d = nc.dram_tensor("cvec", [128, 3 * C], f32, kind=big)
    out_d = nc.dram_tensor(
        "out", [npad, C], f32,
        kind="Internal" if timing_mode else "ExternalOutput")
    dummy_d = (nc.dram_tensor("tdummy", [128, 1], f32, kind="ExternalOutput")
               if timing_mode else None)

    with tile.TileContext(nc) as tc:
        with (
            tc.tile_pool(name="const", bufs=1) as cpool,
            tc.tile_pool(name="gt", bufs=3) as gpool,
            tc.tile_pool(name="sS", bufs=6) as spool,
            tc.tile_pool(name="work", bufs=3) as wpool,
            tc.tile_pool(name="stat", bufs=3) as stpool,
            tc.tile_pool(name="acc", bufs=4,
                         space=bass.MemorySpace.PSUM) as apool,
            tc.tile_pool(name="ps2", bufs=2,
                         space=bass.MemorySpace.PSUM) as p2pool,
        ):
            iota_s = cpool.tile([128, 128], gdt)
            wt_s = cpool.tile([C, C], f32)
            idx_s = cpool.tile([128, nslot // 16], i16)
            nrm_s = cpool.tile([128, nchunk], f32)
            dlo_s = cpool.tile([128, nchunk], f32)
            cvec_s = cpool.tile([128, 3 * C], f32)
            eps_s = cpool.tile([128, 1], f32)
            nc.gpsimd.memset(eps_s[:], float(EPS))
            nc.sync.dma_start(out=iota_s[:], in_=iota_d[:])
            nc.sync.dma_start(out=wt_s[:], in_=wt_d[:])
            nc.sync.dma_start(out=idx_s[:], in_=idx_d[:])
            nc.sync.dma_start(out=nrm_s[:], in_=nrm_d[:])
            nc.sync.dma_start(out=dlo_s[:], in_=dlo_d[:])
            nc.sync.dma_start(out=cvec_s[:], in_=cvec_d[:])

            import contextlib
            loop_cm = (tc.For_i(0, repeat, 1) if repeat > 1
                       else contextlib.nullcontext())
            with loop_cm:
                q = 0  # global chunk cursor
                gather_i = 0
                for g, tiles in enumerate(groups):
                    t0 = tiles[0]
                    ntg = len(tiles)
                    acc = [apool.tile([128, 512], f32, tag="acc", name=f"acc{g}_{i}")
                           for i in range((ntg + 3) // 4)]
                    # gather + accumulate for this group
                    gbatches = [bt for bt in batches if bt[0] == g]
                    for (_, b, s0, ns) in gbatches:
                        nch = ns // 128
                        win = min(N - b * SUB, SUB)
                        gt = gpool.tile([128, maxch, 128], gdt, tag="gt")
                        if not no_gather:
                            nc.gpsimd.dma_gather(
                                gt[:, :nch, :],
                                xtab_d[b * SUB:b * SUB + win, :],
                                idx_s[:, s0 // 16:(s0 + ns) // 16],
                                num_idxs=ns,
                                num_idxs_reg=ns,
                                elem_size=C,
                                queue_num=gather_i % nqueues,
                                single_packet=spkt,
                            )
                        gather_i += 1
                        if only_gather:
                            q += nch
                            continue
                        for ci in range(nch):
                            t = chunk_tile[q]
                            S = spool.tile([128, 128], gdt, tag="sS")
                            nc.vector.tensor_scalar(
                                out=S[:], in0=iota_s[:],
                                scalar1=dlo_s[:, q:q + 1],
                                scalar2=nrm_s[:, q:q + 1],
                                op0=OP.is_equal, op1=OP.mult)
                            j = t - t0
                            nc.tensor.matmul(
                                acc[j // 4][:, (j % 4) * 128:(j % 4) * 128 + 128],
                                gt[:, ci, :], S[:],
                                start=(first_ch[tile_bank[t]] == q),
                                stop=(last_ch[tile_bank[t]] == q))
                            q += 1
                    # transform + LN chain per 4-tile half
                    for h in range(0 if only_gather else (ntg + 3) // 4):
                        hw = min(4, ntg - h * 4)  # tiles in this half
                        W_ = hw * 128
                        aggT = wpool.tile([128, 512], f32, tag="aggT")
                        for j in range(hw):
                            nc.vector.tensor_copy(
                                aggT[:, j * 128:(j + 1) * 128],
                                acc[h][:, j * 128:(j + 1) * 128])
                        ps2 = p2pool.tile([128, 512], f32, tag="ps2")
                        for j in range(hw):
                            nc.tensor.matmul(
                                ps2[:, j * 128:(j + 1) * 128],
                                aggT[:, j * 128:(j + 1) * 128], wt_s[:],
                                start=(j == 0), stop=(j == hw - 1))
                        h1 = wpool.tile([128, 4, 128], f32, tag="h1")
                        if apply_bias:
                            for j in range(hw):
                                nc.vector.tensor_tensor(
                                    out=h1[:, j, :],
                                    in0=ps2[:, j * 128:(j + 1) * 128],
                                    in1=cvec_s[:, 0:C], op=OP.add)
                            nc.scalar.activation(
                                out=h1[:, :hw, :], in_=h1[:, :hw, :], func=AF.Relu)
                        else:
                            nc.scalar.activation(
                                out=h1[:, :hw, :],
                                in_=ps2[:, :W_], func=AF.Relu)
                        xo = wpool.tile([128, 4, 128], f32, tag="xo")
                        r0 = (t0 + h * 4) * 128
                        for j in range(hw):
                            nc.sync.dma_start(
                                out=xo[:, j, :],
                                in_=xown_d[r0 + j * 128:r0 + (j + 1) * 128, :])

                        def layer_norm(dst_t, src_t, gb_off):
                            # per-tile LN over the feature (free) dim
                            s1 = stpool.tile([128, 4], f32, tag="s1")
                            nmu = stpool.tile([128, 4], f32, tag="nmu")
                            ss = stpool.tile([128, 4], f32, tag="ss")
                            sq = wpool.tile([128, 4, 128], f32, tag="sq")
                            std = stpool.tile([128, 4], f32, tag="std")
                            rstd = stpool.tile([128, 4], f32, tag="rstd")
                            nc.vector.tensor_reduce(
                                out=s1[:, :hw], in_=src_t[:, :hw, :],
                                axis=mybir.AxisListType.X, op=OP.add)
                            nc.vector.tensor_scalar_mul(
                                nmu[:, :hw], s1[:, :hw], -1.0 / C)
                            for j in range(hw):
                                nc.scalar.activation(
                                    out=sq[:, j, :], in_=src_t[:, j, :],
                                    func=AF.Square, bias=nmu[:, j:j + 1],
                                    accum_out=ss[:, j:j + 1])
                            nc.scalar.activation(
                                out=std[:, :hw], in_=ss[:, :hw],
                                func=AF.Sqrt, bias=eps_s[:, 0:1], scale=1.0 / C)
                            nc.vector.reciprocal(rstd[:, :hw], std[:, :hw])
                            for j in range(hw):
                                nc.vector.tensor_scalar(
                                    out=dst_t[:, j, :], in0=src_t[:, j, :],
                                    scalar1=nmu[:, j:j + 1],
                                    scalar2=rstd[:, j:j + 1],
                                    op0=OP.add, op1=OP.mult)
                            if gb_off is not None:
                                for j in range(hw):
                                    nc.vector.tensor_tensor(
                                        out=dst_t[:, j, :], in0=dst_t[:, j, :],
                                        in1=cvec_s[:, gb_off:gb_off + C],
                                        op=OP.mult)
                                    nc.vector.tensor_tensor(
                                        out=dst_t[:, j, :], in0=dst_t[:, j, :],
                                        in1=cvec_s[:, gb_off + C:gb_off + 2 * C],
                                        op=OP.add)

                        y1 = wpool.tile([128, 4, 128], f32, tag="y1")
                        layer_norm(y1, h1, C if apply_g1b1 else None)
                        h2 = wpool.tile([128, 4, 128], f32, tag="h2")
                        nc.vector.tensor_tensor(
                            out=h2[:, :hw, :], in0=y1[:, :hw, :],
                            in1=xo[:, :hw, :], op=OP.add)
                        ot = wpool.tile([128, 4, 128], f32, tag="ot")
                        layer_norm(ot, h2, None)
                        if apply_g2b2:
                            # gamma2/beta2 live at cvec offset C (g1b1 unused then)
                            pass
                        for j in range(hw):
                            nc.sync.dma_start(
                                out=out_d[r0 + j * 128:r0 + (j + 1) * 128, :],
                                in_=ot[:, j, :])
                assert q == nchunk
            if dummy_d is not None:
                nc.sync.dma_start(out=dummy_d[:], in_=eps_s[:])
    nc.compile()
    return nc


def _prep(cfg, x, edge_index, W, b, gamma1, beta1, gamma2, beta2):
    import ml_dtypes

    N, C, NCORES = cfg["N"], cfg["C"], cfg["NCORES"]
    npc, ntile, npad, nb, ngrp = _derived(cfg)
    src = np.asarray(edge_index[0], dtype=np.int64)
    dst = np.asarray(edge_index[1], dtype=np.int64)
    x = np.asarray(x, dtype=np.float32)
    W = np.asarray(W, dtype=np.float32)

    deg = (np.bincount(dst, minlength=N) + 1).astype(np.float32)
    dinv = (1.0 / np.sqrt(deg)).astype(np.float32)
    norm = (dinv[src] * dinv[dst]).astype(np.float32)

    sched, cores = _plan(cfg, src, dst, norm, dinv)

    gdt_np = np.float32 if cfg.get("F32TAB") else ml_dtypes.bfloat16
    xtab = np.ascontiguousarray(x.astype(gdt_np))
    wt = np.ascontiguousarray(W.T).astype(np.float32)
    iota = np.ascontiguousarray(np.broadcast_to(
        np.arange(128, dtype=np.float32), (128, 128)).astype(gdt_np))
    cvec = np.zeros((128, 3 * C), dtype=np.float32)
    cvec[:, 0:C] = b
    cvec[:, C:2 * C] = gamma1
    cvec[:, 2 * C:3 * C] = beta1
    # (gamma2/beta2 identity assumed; asserted by caller flags)

    in_maps = []
    for c in range(NCORES):
        xo = np.zeros((npad, C), dtype=np.float32)
        xo[:npc] = x[c * npc:(c + 1) * npc]
        in_maps.append(dict(
            xtab=xtab, xown=xo, wt=wt, iota=iota,
            idx16=cores[c]["idx"], normT=cores[c]["nrm"],
            dstlocT=cores[c]["dlo"], cvec=cvec))
    return sched, in_maps


def _run(cfg, sched, in_maps, apply_bias, apply_g1b1, apply_g2b2, **kw):
    import time

    from concourse.bass_utils import run_bass_kernel_spmd

    t0 = time.time()
    nc = _build_nc(cfg, sched, apply_bias, apply_g1b1, apply_g2b2)
    print(f"[kernel] build+tile-schedule: {time.time() - t0:.1f}s",
          flush=True)
    t0 = time.time()
    res = run_bass_kernel_spmd(
        nc, in_maps, list(range(cfg["NCORES"])), **kw)
    print(f"[kernel] compile+run: {time.time() - t0:.1f}s", flush=True)
    return nc, res


def kernel(x, edge_index, W, b, gamma1, beta1, gamma2, beta2,
           _profile_out=None, _cfg_over=None):
    cfg = _cfg_full()
    if _cfg_over:
        cfg.update(_cfg_over)
    N, C = cfg["N"], cfg["C"]
    npc, ntile, npad, nb, ngrp = _derived(cfg)
    apply_bias = bool(np.any(np.asarray(b)))
    apply_g1b1 = not (np.all(np.asarray(gamma1) == 1)
                      and not np.any(np.asarray(beta1)))
    apply_g2b2 = not (np.all(np.asarray(gamma2) == 1)
                      and not np.any(np.asarray(beta2)))
    assert not apply_g2b2, "general gamma2/beta2 not wired"
    sched, in_maps = _prep(cfg, x, edge_index, W, b,
                           gamma1, beta1, gamma2, beta2)
    kw = {}
    if _profile_out is not None:
        kw = dict(trace=True, tmpdir=_profile_out)
    nc, res = _run(cfg, sched, in_maps, apply_bias, apply_g1b1, apply_g2b2,
                   **kw)
    outs = [res.results[c]["out"][:npc] for c in range(cfg["NCORES"])]
    full = np.concatenate(outs, axis=0).astype(np.float32)
    if _profile_out is not None:
        return full, res
    return full



# revision 8
# speedup vs baseline: 1.4372x; 1.1436x over previous
"""GCN layer (PyG GCNConv + ReLU + LN + residual + LN) on 8 Trainium2 cores.

Math: out = LN2(x + LN1(relu(A_hat @ x @ W.T + b)))  with
A_hat = D^-1/2 (A+I) D^-1/2.  The per-edge weight factorizes
(norm_e = dinv[src]*dinv[dst]) and aggregation commutes with the linear
layer, so each core:
  - gathers raw x rows (bf16) for the edges whose dst it owns (dma_gather)
  - scatter-adds them into per-dst-tile accumulators via one-hot matmuls
    on the PE: S[k, n] = (n == dstloc_k) * norm_k  built by one fused DVE
    tensor_scalar; psumT[feat, node] += g_chunk.T @ S
  - applies W via a second matmul (psum2[node, feat] = aggT.T @ W.T)
  - runs the bias/relu/LN1/residual/LN2 chain on 512-wide tiles.

Host-side numpy does graph preprocessing only: degrees, edge partitioning
by dst, bucketing by src>>15 (int16 gather-index windows), padding to
128-edge chunks, and a static chunk schedule shared by all 8 cores.
"""

import sys

import numpy as np

sys.path.insert(0, "/opt/trn_rl_repo")

EPS = 1e-5


def _cfg_full():
    return dict(
        N=100000,  # nodes
        C=128,  # features
        NCORES=8,
        SUB=32768,  # int16 gather window (rows per sub-table)
        GRP=8,  # dst tiles per psum group
    )


def _derived(cfg):
    N, NCORES = cfg["N"], cfg["NCORES"]
    npc = N // NCORES  # nodes per core
    assert npc * NCORES == N
    ntile = -(-npc // 128)  # dst tiles per core
    npad = ntile * 128
    nb = -(-N // cfg["SUB"])  # src buckets
    ngrp = -(-ntile // cfg["GRP"])
    return npc, ntile, npad, nb, ngrp


def _plan(cfg, src, dst, norm, dinv):
    """Build the shared static schedule + per-core host arrays.

    Returns (sched, cores) where sched has the chunk->tile mapping shared
    by all cores and cores[c] has idx/norm/dstloc arrays for core c.
    """
    N, C, NCORES, SUB, GRP = (
        cfg["N"], cfg["C"], cfg["NCORES"], cfg["SUB"], cfg["GRP"])
    npc, ntile, npad, nb, ngrp = _derived(cfg)
    ncell = ntile * nb

    per_core = []
    counts = np.zeros((NCORES, ncell), dtype=np.int64)
    for c in range(NCORES):
        base = c * npc
        m = (dst >= base) & (dst < base + npc)
        es, ed, en = src[m], dst[m], norm[m]
        # self loops for own nodes: src=dst=v, weight dinv[v]^2
        own = np.arange(base, base + npc, dtype=np.int64)
        es = np.concatenate([es, own])
        ed = np.concatenate([ed, own])
        en = np.concatenate([en, (dinv[own] * dinv[own]).astype(np.float32)])
        t = (ed - base) >> 7
        bkt = es // SUB
        cell = t * nb + bkt
        counts[c] = np.bincount(cell, minlength=ncell)
        per_core.append((es, ed - base, en, cell))

    cap = counts.max(axis=0)  # per (tile,bucket) max edges over cores
    chunks_per_cell = -(-cap // 128)  # 0 if cell empty on all cores
    # chunk schedule: group -> bucket -> tile in group -> chunks
    chunk_tile = []  # global chunk -> tile id
    cell_slot0 = np.zeros(ncell, dtype=np.int64)  # cell -> first slot
    batches = []  # (bucket, slot0, nslots) per gather instruction
    groups = []  # list of lists of tile ids
    slot = 0
    for g in range(ngrp):
        tiles = list(range(g * GRP, min((g + 1) * GRP, ntile)))
        groups.append(tiles)
        for b in range(nb):
            s0 = slot
            for t in tiles:
                cell = t * nb + b
                nch = int(chunks_per_cell[cell])
                if nch == 0:
                    continue
                cell_slot0[cell] = slot
                chunk_tile.extend([t] * nch)
                slot += nch * 128
            # split into gather instructions of <= bmax indices (the SWDGE
            # descriptor carveout rejects much larger single instructions)
            bmax = cfg.get("BMAX", 896)
            p = s0
            while p < slot:
                ns = min(bmax, slot - p)
                batches.append((g, b, p, ns))
                p += ns
    nslot = slot
    nchunk = nslot // 128
    assert nslot % 128 == 0

    cores = []
    for c in range(NCORES):
        es, dloc, en, cell = per_core[c]
        idx = np.zeros(nslot, dtype=np.int16)
        nrm = np.zeros(nslot, dtype=np.float32)
        dlo = np.zeros(nslot, dtype=np.float32)
        order = np.argsort(cell, kind="stable")
        cell_sorted = cell[order]
        # rank within cell
        cnt = counts[c]
        starts = np.zeros(ncell, dtype=np.int64)
        np.cumsum(cnt[:-1], out=starts[1:])
        rank = np.arange(len(order)) - starts[cell_sorted]
        pos = cell_slot0[cell_sorted] + rank
        idx[pos] = (es[order] - (cell_sorted % nb) * SUB).astype(np.int16)
        nrm[pos] = en[order]
        dlo[pos] = (dloc[order] & 127).astype(np.float32)
        # wrap indices into 16 partitions, replicate to 128
        idx_t = np.ascontiguousarray(
            np.tile(idx.reshape(-1, 16).T, (8, 1)))  # [128, nslot//16]
        nrm_t = np.ascontiguousarray(nrm.reshape(-1, 128).T)  # [128, nchunk]
        dlo_t = np.ascontiguousarray(dlo.reshape(-1, 128).T)
        cores.append(dict(idx=idx_t, nrm=nrm_t, dlo=dlo_t))

    sched = dict(chunk_tile=chunk_tile, batches=batches, groups=groups,
                 nslot=nslot, nchunk=nchunk, ntile=ntile, nb=nb)
    return sched, cores


def _build_nc(cfg, sched, apply_bias, apply_g1b1, apply_g2b2, repeat=1,
              timing_mode=False):
    import concourse.bass as bass
    import concourse.bacc as bacc
    import concourse.mybir as mybir
    import concourse.tile as tile

    N, C, SUB, GRP = cfg["N"], cfg["C"], cfg["SUB"], cfg["GRP"]
    npc, ntile, npad, nb, ngrp = _derived(cfg)
    nslot, nchunk = sched["nslot"], sched["nchunk"]
    chunk_tile, batches, groups = (
        sched["chunk_tile"], sched["batches"], sched["groups"])
    f32, bf16, i16 = mybir.dt.float32, mybir.dt.bfloat16, mybir.dt.int16
    AF = mybir.ActivationFunctionType
    OP = mybir.AluOpType

    # first/last chunk index per psum bank (= up to 4 dst tiles of one
    # group); start=True zeroes a whole 2KB zero-region, so flags are
    # per bank
    tile_bank = {}
    for g, tiles in enumerate(groups):
        for t in tiles:
            tile_bank[t] = (g, (t - tiles[0]) // 4)
    first_ch, last_ch = {}, {}
    for q, t in enumerate(chunk_tile):
        bank = tile_bank[t]
        if bank not in first_ch:
            first_ch[bank] = q
        last_ch[bank] = q

    maxch = max(ns // 128 for (_, _, _, ns) in batches)

    only_gather = cfg.get("ONLY_GATHER", False)
    no_gather = cfg.get("NO_GATHER", False)
    f32tab = cfg.get("F32TAB", False)
    spkt = cfg.get("SINGLE_PACKET", True)
    nqueues = cfg.get("QUEUES", 1)
    nc = bacc.Bacc("TRN2", target_bir_lowering=False, debug=False,
                   dynamic_dma_scratch_size=cfg.get("SCRATCH", 16384),
                   num_swdge_queues=nqueues)
    # timing_mode: only idx16 (drives gather addresses) stays external;
    # value-only tensors become internal DRAM so per-call host transfers
    # shrink from ~260MB to ~30MB
    big = "Internal" if timing_mode else "ExternalInput"
    gdt = f32 if f32tab else bf16
    xtab_d = nc.dram_tensor("xtab", [N, C], gdt, kind=big)
    xown_d = nc.dram_tensor("xown", [npad, C], f32, kind=big)
    wt_d = nc.dram_tensor("wt", [C, C], f32, kind=big)
    iota_d = nc.dram_tensor("iota", [128, 128], gdt, kind=big)
    idx_d = nc.dram_tensor("idx16", [128, nslot // 16], i16,
                           kind="ExternalInput")
    sdt = gdt if cfg.get("SBATCH") else f32
    nrm_d = nc.dram_tensor("normT", [128, nchunk], sdt, kind=big)
    dlo_d = nc.dram_tensor("dstlocT", [128, nchunk], sdt, kind=big)
    cvec_d = nc.dram_tensor("cvec", [128, 3 * C], f32, kind=big)
    out_d = nc.dram_tensor(
        "out", [npad, C], f32,
        kind="Internal" if timing_mode else "ExternalOutput")
    dummy_d = (nc.dram_tensor("tdummy", [128, 1], f32, kind="ExternalOutput")
               if timing_mode else None)

    SBATCH = cfg.get("SBATCH", 0)  # chunks per batched S-build (0 = per-chunk)
    with tile.TileContext(nc) as tc:
        with (
            tc.tile_pool(name="const", bufs=1) as cpool,
            tc.tile_pool(name="gt", bufs=3) as gpool,
            tc.tile_pool(name="sS", bufs=(6 if not SBATCH else 1)) as spool,
            tc.tile_pool(name="sbig", bufs=6) as sbpool,
            tc.tile_pool(name="work", bufs=3) as wpool,
            tc.tile_pool(name="stat", bufs=3) as stpool,
            tc.tile_pool(name="acc", bufs=4,
                         space=bass.MemorySpace.PSUM) as apool,
            tc.tile_pool(name="ps2", bufs=2,
                         space=bass.MemorySpace.PSUM) as p2pool,
        ):
            iota_s = cpool.tile([128, 128], gdt)
            wt_s = cpool.tile([C, C], f32)
            idx_s = cpool.tile([128, nslot // 16], i16)
            nrm_s = cpool.tile([128, nchunk], sdt)
            dlo_s = cpool.tile([128, nchunk], sdt)
            cvec_s = cpool.tile([128, 3 * C], f32)
            eps_s = cpool.tile([128, 1], f32)
            nc.gpsimd.memset(eps_s[:], float(EPS))
            nc.sync.dma_start(out=iota_s[:], in_=iota_d[:])
            nc.sync.dma_start(out=wt_s[:], in_=wt_d[:])
            nc.sync.dma_start(out=idx_s[:], in_=idx_d[:])
            nc.sync.dma_start(out=nrm_s[:], in_=nrm_d[:])
            nc.sync.dma_start(out=dlo_s[:], in_=dlo_d[:])
            nc.sync.dma_start(out=cvec_s[:], in_=cvec_d[:])

            import contextlib
            loop_cm = (tc.For_i(0, repeat, 1) if repeat > 1
                       else contextlib.nullcontext())
            with loop_cm:
                q = 0  # global chunk cursor
                gather_i = 0
                for g, tiles in enumerate(groups):
                    t0 = tiles[0]
                    ntg = len(tiles)
                    acc = [apool.tile([128, 512], f32, tag="acc", name=f"acc{g}_{i}")
                           for i in range((ntg + 3) // 4)]
                    # gather + accumulate for this group
                    gbatches = [bt for bt in batches if bt[0] == g]
                    s_slice = {}
                    if SBATCH:
                        # batched S build: 2 DVE ops per section instead of
                        # one tensor_scalar per chunk
                        gnch = sum(bns // 128 for (_, _, _, bns) in gbatches)
                        for sq0 in range(q, q + gnch, SBATCH):
                            k = min(SBATCH, q + gnch - sq0)
                            sb = sbpool.tile([128, SBATCH, 128], gdt,
                                             tag="sbig")
                            iota_b = iota_s[:].unsqueeze(1).broadcast_to(
                                [128, k, 128])
                            dlo_b = dlo_s[:, sq0:sq0 + k].unsqueeze(
                                2).broadcast_to([128, k, 128])
                            nrm_b = nrm_s[:, sq0:sq0 + k].unsqueeze(
                                2).broadcast_to([128, k, 128])
                            nc.vector.tensor_tensor(
                                out=sb[:, :k, :], in0=iota_b, in1=dlo_b,
                                op=OP.is_equal)
                            nc.vector.tensor_tensor(
                                out=sb[:, :k, :], in0=sb[:, :k, :],
                                in1=nrm_b, op=OP.mult)
                            for qq in range(sq0, sq0 + k):
                                s_slice[qq] = (sb, qq - sq0)
                    for (_, b, s0, ns) in gbatches:
                        nch = ns // 128
                        win = min(N - b * SUB, SUB)
                        gt = gpool.tile([128, maxch, 128], gdt, tag="gt")
                        if not no_gather:
                            nc.gpsimd.dma_gather(
                                gt[:, :nch, :],
                                xtab_d[b * SUB:b * SUB + win, :],
                                idx_s[:, s0 // 16:(s0 + ns) // 16],
                                num_idxs=ns,
                                num_idxs_reg=ns,
                                elem_size=C,
                                queue_num=gather_i % nqueues,
                                single_packet=spkt,
                            )
                        gather_i += 1
                        if only_gather:
                            q += nch
                            continue
                        for ci in range(nch):
                            t = chunk_tile[q]
                            if SBATCH:
                                sb, off = s_slice[q]
                                S_ap = sb[:, off, :]
                            else:
                                S = spool.tile([128, 128], gdt, tag="sS")
                                nc.vector.tensor_scalar(
                                    out=S[:], in0=iota_s[:],
                                    scalar1=dlo_s[:, q:q + 1],
                                    scalar2=nrm_s[:, q:q + 1],
                                    op0=OP.is_equal, op1=OP.mult)
                                S_ap = S[:]
                            j = t - t0
                            nc.tensor.matmul(
                                acc[j // 4][:, (j % 4) * 128:(j % 4) * 128 + 128],
                                gt[:, ci, :], S_ap,
                                start=(first_ch[tile_bank[t]] == q),
                                stop=(last_ch[tile_bank[t]] == q))
                            q += 1
                    # transform + LN chain per 4-tile half
                    for h in range(0 if only_gather else (ntg + 3) // 4):
                        hw = min(4, ntg - h * 4)  # tiles in this half
                        W_ = hw * 128
                        aggT = wpool.tile([128, 512], f32, tag="aggT")
                        for j in range(hw):
                            nc.vector.tensor_copy(
                                aggT[:, j * 128:(j + 1) * 128],
                                acc[h][:, j * 128:(j + 1) * 128])
                        ps2 = p2pool.tile([128, 512], f32, tag="ps2")
                        for j in range(hw):
                            nc.tensor.matmul(
                                ps2[:, j * 128:(j + 1) * 128],
                                aggT[:, j * 128:(j + 1) * 128], wt_s[:],
                                start=(j == 0), stop=(j == hw - 1))
                        h1 = wpool.tile([128, 4, 128], f32, tag="h1")
                        if apply_bias:
                            for j in range(hw):
                                nc.vector.tensor_tensor(
                                    out=h1[:, j, :],
                                    in0=ps2[:, j * 128:(j + 1) * 128],
                                    in1=cvec_s[:, 0:C], op=OP.add)
                            nc.scalar.activation(
                                out=h1[:, :hw, :], in_=h1[:, :hw, :], func=AF.Relu)
                        else:
                            nc.scalar.activation(
                                out=h1[:, :hw, :],
                                in_=ps2[:, :W_], func=AF.Relu)
                        xo = wpool.tile([128, 4, 128], f32, tag="xo")
                        r0 = (t0 + h * 4) * 128
                        for j in range(hw):
                            nc.sync.dma_start(
                                out=xo[:, j, :],
                                in_=xown_d[r0 + j * 128:r0 + (j + 1) * 128, :])

                        def layer_norm(dst_t, src_t, gb_off):
                            # per-tile LN over the feature (free) dim
                            s1 = stpool.tile([128, 4], f32, tag="s1")
                            nmu = stpool.tile([128, 4], f32, tag="nmu")
                            ss = stpool.tile([128, 4], f32, tag="ss")
                            sq = wpool.tile([128, 4, 128], f32, tag="sq")
                            std = stpool.tile([128, 4], f32, tag="std")
                            rstd = stpool.tile([128, 4], f32, tag="rstd")
                            nc.vector.tensor_reduce(
                                out=s1[:, :hw], in_=src_t[:, :hw, :],
                                axis=mybir.AxisListType.X, op=OP.add)
                            nc.vector.tensor_scalar_mul(
                                nmu[:, :hw], s1[:, :hw], -1.0 / C)
                            for j in range(hw):
                                nc.scalar.activation(
                                    out=sq[:, j, :], in_=src_t[:, j, :],
                                    func=AF.Square, bias=nmu[:, j:j + 1],
                                    accum_out=ss[:, j:j + 1])
                            nc.scalar.activation(
                                out=std[:, :hw], in_=ss[:, :hw],
                                func=AF.Sqrt, bias=eps_s[:, 0:1], scale=1.0 / C)
                            nc.vector.reciprocal(rstd[:, :hw], std[:, :hw])
                            for j in range(hw):
                                nc.vector.tensor_scalar(
                                    out=dst_t[:, j, :], in0=src_t[:, j, :],
                                    scalar1=nmu[:, j:j + 1],
                                    scalar2=rstd[:, j:j + 1],
                                    op0=OP.add, op1=OP.mult)
                            if gb_off is not None:
                                for j in range(hw):
                                    nc.vector.tensor_tensor(
                                        out=dst_t[:, j, :], in0=dst_t[:, j, :],
                                        in1=cvec_s[:, gb_off:gb_off + C],
                                        op=OP.mult)
                                    nc.vector.tensor_tensor(
                                        out=dst_t[:, j, :], in0=dst_t[:, j, :],
                                        in1=cvec_s[:, gb_off + C:gb_off + 2 * C],
                                        op=OP.add)

                        y1 = wpool.tile([128, 4, 128], f32, tag="y1")
                        layer_norm(y1, h1, C if apply_g1b1 else None)
                        h2 = wpool.tile([128, 4, 128], f32, tag="h2")
                        nc.vector.tensor_tensor(
                            out=h2[:, :hw, :], in0=y1[:, :hw, :],
                            in1=xo[:, :hw, :], op=OP.add)
                        ot = wpool.tile([128, 4, 128], f32, tag="ot")
                        layer_norm(ot, h2, None)
                        if apply_g2b2:
                            # gamma2/beta2 live at cvec offset C (g1b1 unused then)
                            pass
                        for j in range(hw):
                            nc.sync.dma_start(
                                out=out_d[r0 + j * 128:r0 + (j + 1) * 128, :],
                                in_=ot[:, j, :])
                assert q == nchunk
            if dummy_d is not None:
                nc.sync.dma_start(out=dummy_d[:], in_=eps_s[:])
    nc.compile()
    return nc


def _prep(cfg, x, edge_index, W, b, gamma1, beta1, gamma2, beta2):
    import ml_dtypes

    N, C, NCORES = cfg["N"], cfg["C"], cfg["NCORES"]
    npc, ntile, npad, nb, ngrp = _derived(cfg)
    src = np.asarray(edge_index[0], dtype=np.int64)
    dst = np.asarray(edge_index[1], dtype=np.int64)
    x = np.asarray(x, dtype=np.float32)
    W = np.asarray(W, dtype=np.float32)

    deg = (np.bincount(dst, minlength=N) + 1).astype(np.float32)
    dinv = (1.0 / np.sqrt(deg)).astype(np.float32)
    norm = (dinv[src] * dinv[dst]).astype(np.float32)

    sched, cores = _plan(cfg, src, dst, norm, dinv)

    gdt_np = np.float32 if cfg.get("F32TAB") else ml_dtypes.bfloat16
    xtab = np.ascontiguousarray(x.astype(gdt_np))
    wt = np.ascontiguousarray(W.T).astype(np.float32)
    iota = np.ascontiguousarray(np.broadcast_to(
        np.arange(128, dtype=np.float32), (128, 128)).astype(gdt_np))
    cvec = np.zeros((128, 3 * C), dtype=np.float32)
    cvec[:, 0:C] = b
    cvec[:, C:2 * C] = gamma1
    cvec[:, 2 * C:3 * C] = beta1
    # (gamma2/beta2 identity assumed; asserted by caller flags)

    sdt_np = gdt_np if cfg.get("SBATCH") else np.float32
    in_maps = []
    for c in range(NCORES):
        xo = np.zeros((npad, C), dtype=np.float32)
        xo[:npc] = x[c * npc:(c + 1) * npc]
        in_maps.append(dict(
            xtab=xtab, xown=xo, wt=wt, iota=iota,
            idx16=cores[c]["idx"],
            normT=np.ascontiguousarray(cores[c]["nrm"].astype(sdt_np)),
            dstlocT=np.ascontiguousarray(cores[c]["dlo"].astype(sdt_np)),
            cvec=cvec))
    return sched, in_maps


def _run(cfg, sched, in_maps, apply_bias, apply_g1b1, apply_g2b2, **kw):
    import time

    from concourse.bass_utils import run_bass_kernel_spmd

    t0 = time.time()
    nc = _build_nc(cfg, sched, apply_bias, apply_g1b1, apply_g2b2)
    print(f"[kernel] build+tile-schedule: {time.time() - t0:.1f}s",
          flush=True)
    t0 = time.time()
    res = run_bass_kernel_spmd(
        nc, in_maps, list(range(cfg["NCORES"])), **kw)
    print(f"[kernel] compile+run: {time.time() - t0:.1f}s", flush=True)
    return nc, res


def kernel(x, edge_index, W, b, gamma1, beta1, gamma2, beta2,
           _profile_out=None, _cfg_over=None):
    cfg = _cfg_full()
    if _cfg_over:
        cfg.update(_cfg_over)
    N, C = cfg["N"], cfg["C"]
    npc, ntile, npad, nb, ngrp = _derived(cfg)
    apply_bias = bool(np.any(np.asarray(b)))
    apply_g1b1 = not (np.all(np.asarray(gamma1) == 1)
                      and not np.any(np.asarray(beta1)))
    apply_g2b2 = not (np.all(np.asarray(gamma2) == 1)
                      and not np.any(np.asarray(beta2)))
    assert not apply_g2b2, "general gamma2/beta2 not wired"
    sched, in_maps = _prep(cfg, x, edge_index, W, b,
                           gamma1, beta1, gamma2, beta2)
    kw = {}
    if _profile_out is not None:
        kw = dict(trace=True, tmpdir=_profile_out)
    nc, res = _run(cfg, sched, in_maps, apply_bias, apply_g1b1, apply_g2b2,
                   **kw)
    outs = [res.results[c]["out"][:npc] for c in range(cfg["NCORES"])]
    full = np.concatenate(outs, axis=0).astype(np.float32)
    if _profile_out is not None:
        return full, res
    return full



# revision 11
# speedup vs baseline: 1.9921x; 1.3861x over previous
"""GCN layer (PyG GCNConv + ReLU + LN + residual + LN) on 8 Trainium2 cores.

Math: out = LN2(x + LN1(relu(A_hat @ x @ W.T + b)))  with
A_hat = D^-1/2 (A+I) D^-1/2.  The per-edge weight factorizes
(norm_e = dinv[src]*dinv[dst]) and aggregation commutes with the linear
layer, so each core:
  - gathers raw x rows (bf16) for the edges whose dst it owns (dma_gather)
  - scatter-adds them into per-dst-tile accumulators via one-hot matmuls
    on the PE: S[k, n] = (n == dstloc_k) * norm_k  built by one fused DVE
    tensor_scalar; psumT[feat, node] += g_chunk.T @ S
  - applies W via a second matmul (psum2[node, feat] = aggT.T @ W.T)
  - runs the bias/relu/LN1/residual/LN2 chain on 512-wide tiles.

Host-side numpy does graph preprocessing only: degrees, edge partitioning
by dst, bucketing by src>>15 (int16 gather-index windows), padding to
128-edge chunks, and a static chunk schedule shared by all 8 cores.
"""

import sys

import numpy as np

sys.path.insert(0, "/opt/trn_rl_repo")

EPS = 1e-5


def _cfg_full():
    return dict(
        N=100000,  # nodes
        C=128,  # features
        NCORES=8,
        SUB=32768,  # int16 gather window (rows per sub-table)
        GRP=8,  # dst tiles per psum group
    )


def _derived(cfg):
    N, NCORES = cfg["N"], cfg["NCORES"]
    npc = N // NCORES  # nodes per core
    assert npc * NCORES == N
    ntile = -(-npc // 128)  # dst tiles per core
    npad = ntile * 128
    nb = -(-N // cfg["SUB"])  # src buckets
    ngrp = -(-ntile // cfg["GRP"])
    return npc, ntile, npad, nb, ngrp


def _plan(cfg, src, dst, norm, dinv):
    """Build the shared static schedule + per-core host arrays.

    Returns (sched, cores) where sched has the chunk->tile mapping shared
    by all cores and cores[c] has idx/norm/dstloc arrays for core c.
    """
    N, C, NCORES, SUB, GRP = (
        cfg["N"], cfg["C"], cfg["NCORES"], cfg["SUB"], cfg["GRP"])
    npc, ntile, npad, nb, ngrp = _derived(cfg)
    ncell = ntile * nb

    per_core = []
    counts = np.zeros((NCORES, ncell), dtype=np.int64)
    for c in range(NCORES):
        base = c * npc
        m = (dst >= base) & (dst < base + npc)
        es, ed, en = src[m], dst[m], norm[m]
        # self loops for own nodes: src=dst=v, weight dinv[v]^2
        own = np.arange(base, base + npc, dtype=np.int64)
        es = np.concatenate([es, own])
        ed = np.concatenate([ed, own])
        en = np.concatenate([en, (dinv[own] * dinv[own]).astype(np.float32)])
        t = (ed - base) >> 7
        bkt = es // SUB
        cell = t * nb + bkt
        counts[c] = np.bincount(cell, minlength=ncell)
        per_core.append((es, ed - base, en, cell))

    cap = counts.max(axis=0)  # per (tile,bucket) max edges over cores
    chunks_per_cell = -(-cap // 128)  # 0 if cell empty on all cores
    # chunk schedule: group -> bucket -> tile in group -> chunks
    chunk_tile = []  # global chunk -> tile id
    cell_slot0 = np.zeros(ncell, dtype=np.int64)  # cell -> first slot
    batches = []  # (bucket, slot0, nslots) per gather instruction
    groups = []  # list of lists of tile ids
    slot = 0
    for g in range(ngrp):
        tiles = list(range(g * GRP, min((g + 1) * GRP, ntile)))
        groups.append(tiles)
        for b in range(nb):
            s0 = slot
            for t in tiles:
                cell = t * nb + b
                nch = int(chunks_per_cell[cell])
                if nch == 0:
                    continue
                cell_slot0[cell] = slot
                chunk_tile.extend([t] * nch)
                slot += nch * 128
            # split into gather instructions of <= bmax indices (the SWDGE
            # descriptor carveout rejects much larger single instructions)
            bmax = cfg.get("BMAX", 896)
            p = s0
            while p < slot:
                ns = min(bmax, slot - p)
                batches.append((g, b, p, ns))
                p += ns
    nslot = slot
    nchunk = nslot // 128
    assert nslot % 128 == 0

    cores = []
    for c in range(NCORES):
        es, dloc, en, cell = per_core[c]
        idx = np.zeros(nslot, dtype=np.int16)
        nrm = np.zeros(nslot, dtype=np.float32)
        dlo = np.zeros(nslot, dtype=np.float32)
        order = np.argsort(cell, kind="stable")
        cell_sorted = cell[order]
        # rank within cell
        cnt = counts[c]
        starts = np.zeros(ncell, dtype=np.int64)
        np.cumsum(cnt[:-1], out=starts[1:])
        rank = np.arange(len(order)) - starts[cell_sorted]
        pos = cell_slot0[cell_sorted] + rank
        idx[pos] = (es[order] - (cell_sorted % nb) * SUB).astype(np.int16)
        nrm[pos] = en[order]
        dlo[pos] = (dloc[order] & 127).astype(np.float32)
        # wrap indices into 16 partitions, replicate to 128
        idx_t = np.ascontiguousarray(
            np.tile(idx.reshape(-1, 16).T, (8, 1)))  # [128, nslot//16]
        nrm_t = np.ascontiguousarray(nrm.reshape(-1, 128).T)  # [128, nchunk]
        dlo_t = np.ascontiguousarray(dlo.reshape(-1, 128).T)
        cores.append(dict(idx=idx_t, nrm=nrm_t, dlo=dlo_t))

    sched = dict(chunk_tile=chunk_tile, batches=batches, groups=groups,
                 nslot=nslot, nchunk=nchunk, ntile=ntile, nb=nb)
    return sched, cores


def _build_nc(cfg, sched, apply_bias, apply_g1b1, apply_g2b2, repeat=1,
              timing_mode=False):
    import concourse.bass as bass
    import concourse.bacc as bacc
    import concourse.mybir as mybir
    import concourse.tile as tile

    N, C, SUB, GRP = cfg["N"], cfg["C"], cfg["SUB"], cfg["GRP"]
    npc, ntile, npad, nb, ngrp = _derived(cfg)
    nslot, nchunk = sched["nslot"], sched["nchunk"]
    chunk_tile, batches, groups = (
        sched["chunk_tile"], sched["batches"], sched["groups"])
    f32, bf16, i16 = mybir.dt.float32, mybir.dt.bfloat16, mybir.dt.int16
    AF = mybir.ActivationFunctionType
    OP = mybir.AluOpType

    # first/last chunk index per psum bank (= up to 4 dst tiles of one
    # group); start=True zeroes a whole 2KB zero-region, so flags are
    # per bank
    tile_bank = {}
    for g, tiles in enumerate(groups):
        for t in tiles:
            tile_bank[t] = (g, (t - tiles[0]) // 4)
    first_ch, last_ch = {}, {}
    for q, t in enumerate(chunk_tile):
        bank = tile_bank[t]
        if bank not in first_ch:
            first_ch[bank] = q
        last_ch[bank] = q

    maxch = max(ns // 128 for (_, _, _, ns) in batches)

    only_gather = cfg.get("ONLY_GATHER", False)
    no_gather = cfg.get("NO_GATHER", False)
    f32tab = cfg.get("F32TAB", False)
    spkt = cfg.get("SINGLE_PACKET", True)
    nqueues = cfg.get("QUEUES", 1)
    nc = bacc.Bacc("TRN2", target_bir_lowering=False, debug=False,
                   dynamic_dma_scratch_size=cfg.get("SCRATCH", 16384),
                   num_swdge_queues=nqueues)
    # timing_mode: only idx16 (drives gather addresses) stays external;
    # value-only tensors become internal DRAM so per-call host transfers
    # shrink from ~260MB to ~30MB
    big = "Internal" if timing_mode else "ExternalInput"
    gdt = f32 if f32tab else bf16
    xtab_d = nc.dram_tensor("xtab", [N, C], gdt, kind=big)
    xown_d = nc.dram_tensor("xown", [npad, C], f32, kind=big)
    wt_d = nc.dram_tensor("wt", [C, C], f32, kind=big)
    iota_d = nc.dram_tensor("iota", [128, 128], gdt, kind=big)
    idx_d = nc.dram_tensor("idx16", [128, nslot // 16], i16,
                           kind="ExternalInput")
    sdt = gdt if cfg.get("SBATCH") else f32
    nrm_d = nc.dram_tensor("normT", [128, nchunk], sdt, kind=big)
    dlo_d = nc.dram_tensor("dstlocT", [128, nchunk], sdt, kind=big)
    cvec_d = nc.dram_tensor("cvec", [128, 3 * C], f32, kind=big)
    out_d = nc.dram_tensor(
        "out", [npad, C], f32,
        kind="Internal" if timing_mode else "ExternalOutput")
    dummy_d = (nc.dram_tensor("tdummy", [128, 1], f32, kind="ExternalOutput")
               if timing_mode else None)

    SBATCH = cfg.get("SBATCH", 0)  # chunks per batched S-build (0 = per-chunk)
    with tile.TileContext(nc) as tc:
        with (
            tc.tile_pool(name="const", bufs=1) as cpool,
            tc.tile_pool(name="gt", bufs=cfg.get("GTBUFS", 3)) as gpool,
            tc.tile_pool(name="sS", bufs=(6 if not SBATCH else 1)) as spool,
            tc.tile_pool(name="sbig", bufs=cfg.get("SBBUFS", 6)) as sbpool,
            tc.tile_pool(name="work", bufs=cfg.get("WBUFS", 3)) as wpool,
            tc.tile_pool(name="stat", bufs=3) as stpool,
            tc.tile_pool(name="acc", bufs=4,
                         space=bass.MemorySpace.PSUM) as apool,
            tc.tile_pool(name="ps2", bufs=2,
                         space=bass.MemorySpace.PSUM) as p2pool,
        ):
            iota_s = cpool.tile([128, 128], gdt)
            wt_s = cpool.tile([C, C], f32)
            idx_s = cpool.tile([128, nslot // 16], i16)
            nrm_s = cpool.tile([128, nchunk], sdt)
            dlo_s = cpool.tile([128, nchunk], sdt)
            cvec_s = cpool.tile([128, 3 * C], f32)
            eps_s = cpool.tile([128, 1], f32)
            nc.gpsimd.memset(eps_s[:], float(EPS))
            nc.sync.dma_start(out=iota_s[:], in_=iota_d[:])
            nc.sync.dma_start(out=wt_s[:], in_=wt_d[:])
            nc.sync.dma_start(out=idx_s[:], in_=idx_d[:])
            nc.sync.dma_start(out=nrm_s[:], in_=nrm_d[:])
            nc.sync.dma_start(out=dlo_s[:], in_=dlo_d[:])
            nc.sync.dma_start(out=cvec_s[:], in_=cvec_d[:])

            import contextlib
            loop_cm = (tc.For_i(0, repeat, 1) if repeat > 1
                       else contextlib.nullcontext())
            with loop_cm:
                q = 0  # global chunk cursor
                gather_i = 0
                for g, tiles in enumerate(groups):
                    t0 = tiles[0]
                    ntg = len(tiles)
                    acc = [apool.tile([128, 512], f32, tag="acc", name=f"acc{g}_{i}")
                           for i in range((ntg + 3) // 4)]
                    # gather + accumulate for this group
                    gbatches = [bt for bt in batches if bt[0] == g]
                    s_slice = {}
                    if SBATCH:
                        # batched S build: 2 DVE ops per section instead of
                        # one tensor_scalar per chunk
                        gnch = sum(bns // 128 for (_, _, _, bns) in gbatches)
                        for sq0 in range(q, q + gnch, SBATCH):
                            k = min(SBATCH, q + gnch - sq0)
                            sb = sbpool.tile([128, SBATCH, 128], gdt,
                                             tag="sbig")
                            iota_b = iota_s[:].unsqueeze(1).broadcast_to(
                                [128, k, 128])
                            dlo_b = dlo_s[:, sq0:sq0 + k].unsqueeze(
                                2).broadcast_to([128, k, 128])
                            nrm_b = nrm_s[:, sq0:sq0 + k].unsqueeze(
                                2).broadcast_to([128, k, 128])
                            nc.vector.tensor_tensor(
                                out=sb[:, :k, :], in0=iota_b, in1=dlo_b,
                                op=OP.is_equal)
                            nc.vector.tensor_tensor(
                                out=sb[:, :k, :], in0=sb[:, :k, :],
                                in1=nrm_b, op=OP.mult)
                            for qq in range(sq0, sq0 + k):
                                s_slice[qq] = (sb, qq - sq0)
                    for (_, b, s0, ns) in gbatches:
                        nch = ns // 128
                        win = min(N - b * SUB, SUB)
                        gt = gpool.tile([128, maxch, 128], gdt, tag="gt")
                        if not no_gather:
                            nc.gpsimd.dma_gather(
                                gt[:, :nch, :],
                                xtab_d[b * SUB:b * SUB + win, :],
                                idx_s[:, s0 // 16:(s0 + ns) // 16],
                                num_idxs=ns,
                                num_idxs_reg=ns,
                                elem_size=C,
                                queue_num=gather_i % nqueues,
                                single_packet=spkt,
                            )
                        gather_i += 1
                        if only_gather:
                            q += nch
                            continue
                        for ci in range(nch):
                            t = chunk_tile[q]
                            if SBATCH:
                                sb, off = s_slice[q]
                                S_ap = sb[:, off, :]
                            else:
                                S = spool.tile([128, 128], gdt, tag="sS")
                                nc.vector.tensor_scalar(
                                    out=S[:], in0=iota_s[:],
                                    scalar1=dlo_s[:, q:q + 1],
                                    scalar2=nrm_s[:, q:q + 1],
                                    op0=OP.is_equal, op1=OP.mult)
                                S_ap = S[:]
                            j = t - t0
                            nc.tensor.matmul(
                                acc[j // 4][:, (j % 4) * 128:(j % 4) * 128 + 128],
                                gt[:, ci, :], S_ap,
                                start=(first_ch[tile_bank[t]] == q),
                                stop=(last_ch[tile_bank[t]] == q))
                            q += 1
                    if cfg.get("LNG") and not only_gather:
                        # group-wide transform + LN chain: stats and applies
                        # batched over all ntg tiles; applies on the Scalar
                        # engine (scale/bias APs), stats via relu accum_out
                        assert not apply_bias and not apply_g1b1
                        h1 = wpool.tile([128, GRP, 128], f32, tag="h1")
                        s1 = stpool.tile([128, GRP], f32, tag="s1")
                        for h in range((ntg + 3) // 4):
                            hw = min(4, ntg - h * 4)
                            aggT = wpool.tile([128, 512], f32, tag="aggT")
                            nc.scalar.activation(
                                out=aggT[:, :hw * 128], in_=acc[h][:, :hw * 128],
                                func=AF.Copy)
                            ps2 = p2pool.tile([128, 512], f32, tag="ps2")
                            for j in range(hw):
                                nc.tensor.matmul(
                                    ps2[:, j * 128:(j + 1) * 128],
                                    aggT[:, j * 128:(j + 1) * 128], wt_s[:],
                                    start=(j == 0), stop=(j == hw - 1))
                            for j in range(hw):
                                jj = h * 4 + j
                                nc.scalar.activation(
                                    out=h1[:, jj, :],
                                    in_=ps2[:, j * 128:(j + 1) * 128],
                                    func=AF.Relu, accum_out=s1[:, jj:jj + 1])
                        xo = wpool.tile([128, GRP, 128], f32, tag="xo")
                        r0 = t0 * 128
                        for j in range(ntg):
                            nc.sync.dma_start(
                                out=xo[:, j, :],
                                in_=xown_d[r0 + j * 128:r0 + (j + 1) * 128, :])

                        def ln_group(dst_t, src_t, s1_t):
                            nmu = stpool.tile([128, GRP], f32, tag="nmu")
                            ss = stpool.tile([128, GRP], f32, tag="ss")
                            std = stpool.tile([128, GRP], f32, tag="std")
                            rstd = stpool.tile([128, GRP], f32, tag="rstd")
                            nm2 = stpool.tile([128, GRP], f32, tag="nm2")
                            sq = wpool.tile([128, 128], f32, tag="sq")
                            nc.vector.tensor_scalar_mul(
                                nmu[:, :ntg], s1_t[:, :ntg], -1.0 / C)
                            for j in range(ntg):
                                nc.scalar.activation(
                                    out=sq[:], in_=src_t[:, j, :],
                                    func=AF.Square, bias=nmu[:, j:j + 1],
                                    accum_out=ss[:, j:j + 1])
                            nc.scalar.activation(
                                out=std[:, :ntg], in_=ss[:, :ntg],
                                func=AF.Sqrt, bias=eps_s[:, 0:1], scale=1.0 / C)
                            nc.vector.reciprocal(rstd[:, :ntg], std[:, :ntg])
                            nc.vector.tensor_mul(
                                nm2[:, :ntg], nmu[:, :ntg], rstd[:, :ntg])
                            for j in range(ntg):
                                nc.scalar.activation(
                                    out=dst_t[:, j, :], in_=src_t[:, j, :],
                                    func=AF.Identity, scale=rstd[:, j:j + 1],
                                    bias=nm2[:, j:j + 1])

                        y1 = wpool.tile([128, GRP, 128], f32, tag="y1")
                        ln_group(y1, h1, s1)
                        h2 = h1  # h1 fully consumed; reuse storage
                        nc.vector.tensor_tensor(
                            out=h2[:, :ntg, :], in0=y1[:, :ntg, :],
                            in1=xo[:, :ntg, :], op=OP.add)
                        s1b = stpool.tile([128, GRP], f32, tag="s1b")
                        nc.vector.tensor_reduce(
                            out=s1b[:, :ntg], in_=h2[:, :ntg, :],
                            axis=mybir.AxisListType.X, op=OP.add)
                        ot = y1  # y1 fully consumed; reuse storage
                        ln_group(ot, h2, s1b)
                        for j in range(ntg):
                            nc.sync.dma_start(
                                out=out_d[r0 + j * 128:r0 + (j + 1) * 128, :],
                                in_=ot[:, j, :])
                    # transform + LN chain per 4-tile half
                    for h in range(0 if (only_gather or cfg.get("LNG"))
                                   else (ntg + 3) // 4):
                        hw = min(4, ntg - h * 4)  # tiles in this half
                        W_ = hw * 128
                        aggT = wpool.tile([128, 512], f32, tag="aggT")
                        for j in range(hw):
                            nc.vector.tensor_copy(
                                aggT[:, j * 128:(j + 1) * 128],
                                acc[h][:, j * 128:(j + 1) * 128])
                        ps2 = p2pool.tile([128, 512], f32, tag="ps2")
                        for j in range(hw):
                            nc.tensor.matmul(
                                ps2[:, j * 128:(j + 1) * 128],
                                aggT[:, j * 128:(j + 1) * 128], wt_s[:],
                                start=(j == 0), stop=(j == hw - 1))
                        h1 = wpool.tile([128, 4, 128], f32, tag="h1")
                        if apply_bias:
                            for j in range(hw):
                                nc.vector.tensor_tensor(
                                    out=h1[:, j, :],
                                    in0=ps2[:, j * 128:(j + 1) * 128],
                                    in1=cvec_s[:, 0:C], op=OP.add)
                            nc.scalar.activation(
                                out=h1[:, :hw, :], in_=h1[:, :hw, :], func=AF.Relu)
                        else:
                            nc.scalar.activation(
                                out=h1[:, :hw, :],
                                in_=ps2[:, :W_], func=AF.Relu)
                        xo = wpool.tile([128, 4, 128], f32, tag="xo")
                        r0 = (t0 + h * 4) * 128
                        for j in range(hw):
                            nc.sync.dma_start(
                                out=xo[:, j, :],
                                in_=xown_d[r0 + j * 128:r0 + (j + 1) * 128, :])

                        def layer_norm(dst_t, src_t, gb_off):
                            # per-tile LN over the feature (free) dim
                            s1 = stpool.tile([128, 4], f32, tag="s1")
                            nmu = stpool.tile([128, 4], f32, tag="nmu")
                            ss = stpool.tile([128, 4], f32, tag="ss")
                            sq = wpool.tile([128, 4, 128], f32, tag="sq")
                            std = stpool.tile([128, 4], f32, tag="std")
                            rstd = stpool.tile([128, 4], f32, tag="rstd")
                            nc.vector.tensor_reduce(
                                out=s1[:, :hw], in_=src_t[:, :hw, :],
                                axis=mybir.AxisListType.X, op=OP.add)
                            nc.vector.tensor_scalar_mul(
                                nmu[:, :hw], s1[:, :hw], -1.0 / C)
                            for j in range(hw):
                                nc.scalar.activation(
                                    out=sq[:, j, :], in_=src_t[:, j, :],
                                    func=AF.Square, bias=nmu[:, j:j + 1],
                                    accum_out=ss[:, j:j + 1])
                            nc.scalar.activation(
                                out=std[:, :hw], in_=ss[:, :hw],
                                func=AF.Sqrt, bias=eps_s[:, 0:1], scale=1.0 / C)
                            nc.vector.reciprocal(rstd[:, :hw], std[:, :hw])
                            for j in range(hw):
                                nc.vector.tensor_scalar(
                                    out=dst_t[:, j, :], in0=src_t[:, j, :],
                                    scalar1=nmu[:, j:j + 1],
                                    scalar2=rstd[:, j:j + 1],
                                    op0=OP.add, op1=OP.mult)
                            if gb_off is not None:
                                for j in range(hw):
                                    nc.vector.tensor_tensor(
                                        out=dst_t[:, j, :], in0=dst_t[:, j, :],
                                        in1=cvec_s[:, gb_off:gb_off + C],
                                        op=OP.mult)
                                    nc.vector.tensor_tensor(
                                        out=dst_t[:, j, :], in0=dst_t[:, j, :],
                                        in1=cvec_s[:, gb_off + C:gb_off + 2 * C],
                                        op=OP.add)

                        y1 = wpool.tile([128, 4, 128], f32, tag="y1")
                        layer_norm(y1, h1, C if apply_g1b1 else None)
                        h2 = wpool.tile([128, 4, 128], f32, tag="h2")
                        nc.vector.tensor_tensor(
                            out=h2[:, :hw, :], in0=y1[:, :hw, :],
                            in1=xo[:, :hw, :], op=OP.add)
                        ot = wpool.tile([128, 4, 128], f32, tag="ot")
                        layer_norm(ot, h2, None)
                        if apply_g2b2:
                            # gamma2/beta2 live at cvec offset C (g1b1 unused then)
                            pass
                        for j in range(hw):
                            nc.sync.dma_start(
                                out=out_d[r0 + j * 128:r0 + (j + 1) * 128, :],
                                in_=ot[:, j, :])
                assert q == nchunk
            if dummy_d is not None:
                nc.sync.dma_start(out=dummy_d[:], in_=eps_s[:])
    nc.compile()
    return nc


def _prep(cfg, x, edge_index, W, b, gamma1, beta1, gamma2, beta2):
    import ml_dtypes

    N, C, NCORES = cfg["N"], cfg["C"], cfg["NCORES"]
    npc, ntile, npad, nb, ngrp = _derived(cfg)
    src = np.asarray(edge_index[0], dtype=np.int64)
    dst = np.asarray(edge_index[1], dtype=np.int64)
    x = np.asarray(x, dtype=np.float32)
    W = np.asarray(W, dtype=np.float32)

    deg = (np.bincount(dst, minlength=N) + 1).astype(np.float32)
    dinv = (1.0 / np.sqrt(deg)).astype(np.float32)
    norm = (dinv[src] * dinv[dst]).astype(np.float32)

    sched, cores = _plan(cfg, src, dst, norm, dinv)

    gdt_np = np.float32 if cfg.get("F32TAB") else ml_dtypes.bfloat16
    xtab = np.ascontiguousarray(x.astype(gdt_np))
    wt = np.ascontiguousarray(W.T).astype(np.float32)
    iota = np.ascontiguousarray(np.broadcast_to(
        np.arange(128, dtype=np.float32), (128, 128)).astype(gdt_np))
    cvec = np.zeros((128, 3 * C), dtype=np.float32)
    cvec[:, 0:C] = b
    cvec[:, C:2 * C] = gamma1
    cvec[:, 2 * C:3 * C] = beta1
    # (gamma2/beta2 identity assumed; asserted by caller flags)

    sdt_np = gdt_np if cfg.get("SBATCH") else np.float32
    in_maps = []
    for c in range(NCORES):
        xo = np.zeros((npad, C), dtype=np.float32)
        xo[:npc] = x[c * npc:(c + 1) * npc]
        in_maps.append(dict(
            xtab=xtab, xown=xo, wt=wt, iota=iota,
            idx16=cores[c]["idx"],
            normT=np.ascontiguousarray(cores[c]["nrm"].astype(sdt_np)),
            dstlocT=np.ascontiguousarray(cores[c]["dlo"].astype(sdt_np)),
            cvec=cvec))
    return sched, in_maps


def _run(cfg, sched, in_maps, apply_bias, apply_g1b1, apply_g2b2, **kw):
    import time

    from concourse.bass_utils import run_bass_kernel_spmd

    t0 = time.time()
    nc = _build_nc(cfg, sched, apply_bias, apply_g1b1, apply_g2b2)
    print(f"[kernel] build+tile-schedule: {time.time() - t0:.1f}s",
          flush=True)
    t0 = time.time()
    res = run_bass_kernel_spmd(
        nc, in_maps, list(range(cfg["NCORES"])), **kw)
    print(f"[kernel] compile+run: {time.time() - t0:.1f}s", flush=True)
    return nc, res


def kernel(x, edge_index, W, b, gamma1, beta1, gamma2, beta2,
           _profile_out=None, _cfg_over=None):
    cfg = _cfg_full()
    if _cfg_over:
        cfg.update(_cfg_over)
    N, C = cfg["N"], cfg["C"]
    npc, ntile, npad, nb, ngrp = _derived(cfg)
    apply_bias = bool(np.any(np.asarray(b)))
    apply_g1b1 = not (np.all(np.asarray(gamma1) == 1)
                      and not np.any(np.asarray(beta1)))
    apply_g2b2 = not (np.all(np.asarray(gamma2) == 1)
                      and not np.any(np.asarray(beta2)))
    assert not apply_g2b2, "general gamma2/beta2 not wired"
    sched, in_maps = _prep(cfg, x, edge_index, W, b,
                           gamma1, beta1, gamma2, beta2)
    kw = {}
    if _profile_out is not None:
        kw = dict(trace=True, tmpdir=_profile_out)
    nc, res = _run(cfg, sched, in_maps, apply_bias, apply_g1b1, apply_g2b2,
                   **kw)
    outs = [res.results[c]["out"][:npc] for c in range(cfg["NCORES"])]
    full = np.concatenate(outs, axis=0).astype(np.float32)
    if _profile_out is not None:
        return full, res
    return full



# revision 12
# speedup vs baseline: 2.3369x; 1.1731x over previous
"""GCN layer (PyG GCNConv + ReLU + LN + residual + LN) on 8 Trainium2 cores.

Math: out = LN2(x + LN1(relu(A_hat @ x @ W.T + b)))  with
A_hat = D^-1/2 (A+I) D^-1/2.  The per-edge weight factorizes
(norm_e = dinv[src]*dinv[dst]) and aggregation commutes with the linear
layer, so each core:
  - gathers raw x rows (bf16) for the edges whose dst it owns (dma_gather)
  - scatter-adds them into per-dst-tile accumulators via one-hot matmuls
    on the PE: S[k, n] = (n == dstloc_k) * norm_k  built by one fused DVE
    tensor_scalar; psumT[feat, node] += g_chunk.T @ S
  - applies W via a second matmul (psum2[node, feat] = aggT.T @ W.T)
  - runs the bias/relu/LN1/residual/LN2 chain on 512-wide tiles.

Host-side numpy does graph preprocessing only: degrees, edge partitioning
by dst, bucketing by src>>15 (int16 gather-index windows), padding to
128-edge chunks, and a static chunk schedule shared by all 8 cores.
"""

import sys

import numpy as np

sys.path.insert(0, "/opt/trn_rl_repo")

EPS = 1e-5


def _cfg_full():
    return dict(
        N=100000,  # nodes
        C=128,  # features
        NCORES=8,
        SUB=32768,  # int16 gather window (rows per sub-table)
        GRP=8,  # dst tiles per psum group
    )


def _derived(cfg):
    N, NCORES = cfg["N"], cfg["NCORES"]
    npc = N // NCORES  # nodes per core
    assert npc * NCORES == N
    ntile = -(-npc // 128)  # dst tiles per core
    npad = ntile * 128
    nb = -(-N // cfg["SUB"])  # src buckets
    ngrp = -(-ntile // cfg["GRP"])
    return npc, ntile, npad, nb, ngrp


def _plan(cfg, src, dst, norm, dinv):
    """Build the shared static schedule + per-core host arrays.

    Returns (sched, cores) where sched has the chunk->tile mapping shared
    by all cores and cores[c] has idx/norm/dstloc arrays for core c.
    """
    N, C, NCORES, SUB, GRP = (
        cfg["N"], cfg["C"], cfg["NCORES"], cfg["SUB"], cfg["GRP"])
    npc, ntile, npad, nb, ngrp = _derived(cfg)
    ncell = ntile * nb

    per_core = []
    counts = np.zeros((NCORES, ncell), dtype=np.int64)
    for c in range(NCORES):
        base = c * npc
        m = (dst >= base) & (dst < base + npc)
        es, ed, en = src[m], dst[m], norm[m]
        # self loops for own nodes: src=dst=v, weight dinv[v]^2
        own = np.arange(base, base + npc, dtype=np.int64)
        es = np.concatenate([es, own])
        ed = np.concatenate([ed, own])
        en = np.concatenate([en, (dinv[own] * dinv[own]).astype(np.float32)])
        t = (ed - base) >> 7
        bkt = es // SUB
        cell = t * nb + bkt
        counts[c] = np.bincount(cell, minlength=ncell)
        per_core.append((es, ed - base, en, cell))

    cap = counts.max(axis=0)  # per (tile,bucket) max edges over cores
    chunks_per_cell = -(-cap // 128)  # 0 if cell empty on all cores
    # chunk schedule: group -> bucket -> tile in group -> chunks
    chunk_tile = []  # global chunk -> tile id
    cell_slot0 = np.zeros(ncell, dtype=np.int64)  # cell -> first slot
    batches = []  # (bucket, slot0, nslots) per gather instruction
    groups = []  # list of lists of tile ids
    slot = 0
    for g in range(ngrp):
        tiles = list(range(g * GRP, min((g + 1) * GRP, ntile)))
        groups.append(tiles)
        for b in range(nb):
            s0 = slot
            for t in tiles:
                cell = t * nb + b
                nch = int(chunks_per_cell[cell])
                if nch == 0:
                    continue
                cell_slot0[cell] = slot
                chunk_tile.extend([t] * nch)
                slot += nch * 128
            # split into gather instructions of <= bmax indices (the SWDGE
            # descriptor carveout rejects much larger single instructions)
            bmax = cfg.get("BMAX", 896)
            p = s0
            while p < slot:
                ns = min(bmax, slot - p)
                batches.append((g, b, p, ns))
                p += ns
    nslot = slot
    nchunk = nslot // 128
    assert nslot % 128 == 0

    cores = []
    for c in range(NCORES):
        es, dloc, en, cell = per_core[c]
        idx = np.zeros(nslot, dtype=np.int16)
        nrm = np.zeros(nslot, dtype=np.float32)
        dlo = np.zeros(nslot, dtype=np.float32)
        order = np.argsort(cell, kind="stable")
        cell_sorted = cell[order]
        # rank within cell
        cnt = counts[c]
        starts = np.zeros(ncell, dtype=np.int64)
        np.cumsum(cnt[:-1], out=starts[1:])
        rank = np.arange(len(order)) - starts[cell_sorted]
        pos = cell_slot0[cell_sorted] + rank
        idx[pos] = (es[order] - (cell_sorted % nb) * SUB).astype(np.int16)
        nrm[pos] = en[order]
        dlo[pos] = (dloc[order] & 127).astype(np.float32)
        # wrap indices into 16 partitions, replicate to 128
        idx_t = np.ascontiguousarray(
            np.tile(idx.reshape(-1, 16).T, (8, 1)))  # [128, nslot//16]
        nrm_t = np.ascontiguousarray(nrm.reshape(-1, 128).T)  # [128, nchunk]
        dlo_t = np.ascontiguousarray(dlo.reshape(-1, 128).T)
        cores.append(dict(idx=idx_t, nrm=nrm_t, dlo=dlo_t))

    sched = dict(chunk_tile=chunk_tile, batches=batches, groups=groups,
                 nslot=nslot, nchunk=nchunk, ntile=ntile, nb=nb)
    return sched, cores


def _build_nc(cfg, sched, apply_bias, apply_g1b1, apply_g2b2, repeat=1,
              timing_mode=False):
    import concourse.bass as bass
    import concourse.bacc as bacc
    import concourse.mybir as mybir
    import concourse.tile as tile

    N, C, SUB, GRP = cfg["N"], cfg["C"], cfg["SUB"], cfg["GRP"]
    npc, ntile, npad, nb, ngrp = _derived(cfg)
    nslot, nchunk = sched["nslot"], sched["nchunk"]
    chunk_tile, batches, groups = (
        sched["chunk_tile"], sched["batches"], sched["groups"])
    f32, bf16, i16 = mybir.dt.float32, mybir.dt.bfloat16, mybir.dt.int16
    AF = mybir.ActivationFunctionType
    OP = mybir.AluOpType

    # first/last chunk index per psum bank (= up to 4 dst tiles of one
    # group); start=True zeroes a whole 2KB zero-region, so flags are
    # per bank
    tile_bank = {}
    for g, tiles in enumerate(groups):
        for t in tiles:
            tile_bank[t] = (g, (t - tiles[0]) // 4)
    first_ch, last_ch = {}, {}
    for q, t in enumerate(chunk_tile):
        bank = tile_bank[t]
        if bank not in first_ch:
            first_ch[bank] = q
        last_ch[bank] = q

    maxch = max(ns // 128 for (_, _, _, ns) in batches)

    only_gather = cfg.get("ONLY_GATHER", False)
    no_gather = cfg.get("NO_GATHER", False)
    f32tab = cfg.get("F32TAB", False)
    spkt = cfg.get("SINGLE_PACKET", True)
    nqueues = cfg.get("QUEUES", 1)
    nc = bacc.Bacc("TRN2", target_bir_lowering=False, debug=False,
                   dynamic_dma_scratch_size=cfg.get("SCRATCH", 16384),
                   num_swdge_queues=nqueues)
    # timing_mode: only idx16 (drives gather addresses) stays external;
    # value-only tensors become internal DRAM so per-call host transfers
    # shrink from ~260MB to ~30MB
    big = "Internal" if timing_mode else "ExternalInput"
    gdt = f32 if f32tab else bf16
    xtab_d = nc.dram_tensor("xtab", [N, C], gdt, kind=big)
    xown_d = nc.dram_tensor("xown", [npad, C], f32, kind=big)
    wt_d = nc.dram_tensor("wt", [C, C], f32, kind=big)
    iota_d = nc.dram_tensor("iota", [128, 128], gdt, kind=big)
    idx_d = nc.dram_tensor("idx16", [128, nslot // 16], i16,
                           kind="ExternalInput")
    sdt = gdt if cfg.get("SBATCH") else f32
    nrm_d = nc.dram_tensor("normT", [128, nchunk], sdt, kind=big)
    dlo_d = nc.dram_tensor("dstlocT", [128, nchunk], sdt, kind=big)
    cvec_d = nc.dram_tensor("cvec", [128, 3 * C], f32, kind=big)
    out_d = nc.dram_tensor(
        "out", [npad, C], f32,
        kind="Internal" if timing_mode else "ExternalOutput")
    dummy_d = (nc.dram_tensor("tdummy", [128, 1], f32, kind="ExternalOutput")
               if timing_mode else None)

    SBATCH = cfg.get("SBATCH", 0)  # chunks per batched S-build (0 = per-chunk)
    with tile.TileContext(nc) as tc:
        with (
            tc.tile_pool(name="const", bufs=1) as cpool,
            tc.tile_pool(name="gt", bufs=cfg.get("GTBUFS", 3)) as gpool,
            tc.tile_pool(name="sS", bufs=(6 if not SBATCH else 1)) as spool,
            tc.tile_pool(name="sbig", bufs=cfg.get("SBBUFS", 6)) as sbpool,
            tc.tile_pool(name="work", bufs=cfg.get("WBUFS", 3)) as wpool,
            tc.tile_pool(name="stat", bufs=3) as stpool,
            tc.tile_pool(name="acc", bufs=4,
                         space=bass.MemorySpace.PSUM) as apool,
            tc.tile_pool(name="ps2", bufs=2,
                         space=bass.MemorySpace.PSUM) as p2pool,
        ):
            iota_s = cpool.tile([128, 128], gdt)
            wt_s = cpool.tile([C, C], f32)
            idx_s = cpool.tile([128, nslot // 16], i16)
            nrm_s = cpool.tile([128, nchunk], sdt)
            dlo_s = cpool.tile([128, nchunk], sdt)
            cvec_s = cpool.tile([128, 3 * C], f32)
            eps_s = cpool.tile([128, 1], f32)
            nc.gpsimd.memset(eps_s[:], float(EPS))
            nc.sync.dma_start(out=iota_s[:], in_=iota_d[:])
            nc.sync.dma_start(out=wt_s[:], in_=wt_d[:])
            nc.sync.dma_start(out=idx_s[:], in_=idx_d[:])
            nc.sync.dma_start(out=nrm_s[:], in_=nrm_d[:])
            nc.sync.dma_start(out=dlo_s[:], in_=dlo_d[:])
            nc.sync.dma_start(out=cvec_s[:], in_=cvec_d[:])

            import contextlib
            loop_cm = (tc.For_i(0, repeat, 1) if repeat > 1
                       else contextlib.nullcontext())
            with loop_cm:
                q = 0  # global chunk cursor
                gather_i = 0
                for g, tiles in enumerate(groups):
                    t0 = tiles[0]
                    ntg = len(tiles)
                    acc = [apool.tile([128, 512], f32, tag="acc", name=f"acc{g}_{i}")
                           for i in range((ntg + 3) // 4)]
                    # gather + accumulate for this group
                    gbatches = [bt for bt in batches if bt[0] == g]
                    s_slice = {}
                    if SBATCH:
                        # batched S build: 2 DVE ops per section instead of
                        # one tensor_scalar per chunk
                        gnch = sum(bns // 128 for (_, _, _, bns) in gbatches)
                        for sq0 in range(q, q + gnch, SBATCH):
                            k = min(SBATCH, q + gnch - sq0)
                            sb = sbpool.tile([128, SBATCH, 128], gdt,
                                             tag="sbig")
                            iota_b = iota_s[:].unsqueeze(1).broadcast_to(
                                [128, k, 128])
                            dlo_b = dlo_s[:, sq0:sq0 + k].unsqueeze(
                                2).broadcast_to([128, k, 128])
                            nrm_b = nrm_s[:, sq0:sq0 + k].unsqueeze(
                                2).broadcast_to([128, k, 128])
                            nc.vector.tensor_tensor(
                                out=sb[:, :k, :], in0=iota_b, in1=dlo_b,
                                op=OP.is_equal)
                            nc.vector.tensor_tensor(
                                out=sb[:, :k, :], in0=sb[:, :k, :],
                                in1=nrm_b, op=OP.mult)
                            for qq in range(sq0, sq0 + k):
                                s_slice[qq] = (sb, qq - sq0)
                    for (_, b, s0, ns) in gbatches:
                        nch = ns // 128
                        win = min(N - b * SUB, SUB)
                        gt = gpool.tile([128, maxch, 128], gdt, tag="gt")
                        if not no_gather:
                            nc.gpsimd.dma_gather(
                                gt[:, :nch, :],
                                xtab_d[b * SUB:b * SUB + win, :],
                                idx_s[:, s0 // 16:(s0 + ns) // 16],
                                num_idxs=ns,
                                num_idxs_reg=ns,
                                elem_size=C,
                                queue_num=gather_i % nqueues,
                                single_packet=spkt,
                            )
                        gather_i += 1
                        if only_gather:
                            q += nch
                            continue
                        for ci in range(nch):
                            t = chunk_tile[q]
                            if SBATCH:
                                sb, off = s_slice[q]
                                S_ap = sb[:, off, :]
                            else:
                                S = spool.tile([128, 128], gdt, tag="sS")
                                nc.vector.tensor_scalar(
                                    out=S[:], in0=iota_s[:],
                                    scalar1=dlo_s[:, q:q + 1],
                                    scalar2=nrm_s[:, q:q + 1],
                                    op0=OP.is_equal, op1=OP.mult)
                                S_ap = S[:]
                            j = t - t0
                            nc.tensor.matmul(
                                acc[j // 4][:, (j % 4) * 128:(j % 4) * 128 + 128],
                                gt[:, ci, :], S_ap,
                                start=(first_ch[tile_bank[t]] == q),
                                stop=(last_ch[tile_bank[t]] == q))
                            q += 1
                    if cfg.get("LNG") and not only_gather:
                        # group-wide transform + LN chain: stats and applies
                        # batched over all ntg tiles; applies on the Scalar
                        # engine (scale/bias APs), stats via relu accum_out
                        assert not apply_bias and not apply_g1b1
                        h1 = wpool.tile([128, GRP, 128], f32, tag="h1")
                        s1 = stpool.tile([128, GRP], f32, tag="s1")
                        for h in range((ntg + 3) // 4):
                            hw = min(4, ntg - h * 4)
                            aggT = wpool.tile([128, 512], f32, tag="aggT")
                            nc.scalar.activation(
                                out=aggT[:, :hw * 128], in_=acc[h][:, :hw * 128],
                                func=AF.Copy)
                            ps2 = p2pool.tile([128, 512], f32, tag="ps2")
                            for j in range(hw):
                                nc.tensor.matmul(
                                    ps2[:, j * 128:(j + 1) * 128],
                                    aggT[:, j * 128:(j + 1) * 128], wt_s[:],
                                    start=(j == 0), stop=(j == hw - 1))
                            for j in range(hw):
                                jj = h * 4 + j
                                nc.scalar.activation(
                                    out=h1[:, jj, :],
                                    in_=ps2[:, j * 128:(j + 1) * 128],
                                    func=AF.Relu, accum_out=s1[:, jj:jj + 1])
                        xo = wpool.tile([128, GRP, 128], f32, tag="xo")
                        r0 = t0 * 128
                        for j in range(ntg):
                            nc.sync.dma_start(
                                out=xo[:, j, :],
                                in_=xown_d[r0 + j * 128:r0 + (j + 1) * 128, :])

                        def ln_group(dst_t, src_t, s1_t):
                            # stats via s2 = sum(x^2): no bias dependency, so
                            # the Square pass overlaps the tiny-op stat chain
                            nmu = stpool.tile([128, GRP], f32, tag="nmu")
                            s2 = stpool.tile([128, GRP], f32, tag="s2")
                            mu2 = stpool.tile([128, GRP], f32, tag="mu2")
                            var = stpool.tile([128, GRP], f32, tag="var")
                            std = stpool.tile([128, GRP], f32, tag="std")
                            rstd = stpool.tile([128, GRP], f32, tag="rstd")
                            nm2 = stpool.tile([128, GRP], f32, tag="nm2")
                            sq = wpool.tile([128, 128], f32, tag="sq")
                            for j in range(ntg):
                                nc.scalar.activation(
                                    out=sq[:], in_=src_t[:, j, :],
                                    func=AF.Square,
                                    accum_out=s2[:, j:j + 1])
                            nc.vector.tensor_scalar_mul(
                                nmu[:, :ntg], s1_t[:, :ntg], -1.0 / C)
                            nc.vector.tensor_mul(
                                mu2[:, :ntg], nmu[:, :ntg], nmu[:, :ntg])
                            # var = s2/C - mu^2  (+eps via Sqrt bias)
                            nc.vector.scalar_tensor_tensor(
                                out=var[:, :ntg], in0=s2[:, :ntg],
                                scalar=1.0 / C, in1=mu2[:, :ntg],
                                op0=OP.mult, op1=OP.subtract)
                            nc.scalar.activation(
                                out=std[:, :ntg], in_=var[:, :ntg],
                                func=AF.Sqrt, bias=eps_s[:, 0:1])
                            nc.vector.reciprocal(rstd[:, :ntg], std[:, :ntg])
                            nc.vector.tensor_mul(
                                nm2[:, :ntg], nmu[:, :ntg], rstd[:, :ntg])
                            for j in range(ntg):
                                nc.scalar.activation(
                                    out=dst_t[:, j, :], in_=src_t[:, j, :],
                                    func=AF.Identity, scale=rstd[:, j:j + 1],
                                    bias=nm2[:, j:j + 1])

                        y1 = wpool.tile([128, GRP, 128], f32, tag="y1")
                        ln_group(y1, h1, s1)
                        h2 = h1  # h1 fully consumed; reuse storage
                        nc.vector.tensor_tensor(
                            out=h2[:, :ntg, :], in0=y1[:, :ntg, :],
                            in1=xo[:, :ntg, :], op=OP.add)
                        s1b = stpool.tile([128, GRP], f32, tag="s1b")
                        nc.vector.tensor_reduce(
                            out=s1b[:, :ntg], in_=h2[:, :ntg, :],
                            axis=mybir.AxisListType.X, op=OP.add)
                        ot = y1  # y1 fully consumed; reuse storage
                        ln_group(ot, h2, s1b)
                        for j in range(ntg):
                            nc.sync.dma_start(
                                out=out_d[r0 + j * 128:r0 + (j + 1) * 128, :],
                                in_=ot[:, j, :])
                    # transform + LN chain per 4-tile half
                    for h in range(0 if (only_gather or cfg.get("LNG"))
                                   else (ntg + 3) // 4):
                        hw = min(4, ntg - h * 4)  # tiles in this half
                        W_ = hw * 128
                        aggT = wpool.tile([128, 512], f32, tag="aggT")
                        for j in range(hw):
                            nc.vector.tensor_copy(
                                aggT[:, j * 128:(j + 1) * 128],
                                acc[h][:, j * 128:(j + 1) * 128])
                        ps2 = p2pool.tile([128, 512], f32, tag="ps2")
                        for j in range(hw):
                            nc.tensor.matmul(
                                ps2[:, j * 128:(j + 1) * 128],
                                aggT[:, j * 128:(j + 1) * 128], wt_s[:],
                                start=(j == 0), stop=(j == hw - 1))
                        h1 = wpool.tile([128, 4, 128], f32, tag="h1")
                        if apply_bias:
                            for j in range(hw):
                                nc.vector.tensor_tensor(
                                    out=h1[:, j, :],
                                    in0=ps2[:, j * 128:(j + 1) * 128],
                                    in1=cvec_s[:, 0:C], op=OP.add)
                            nc.scalar.activation(
                                out=h1[:, :hw, :], in_=h1[:, :hw, :], func=AF.Relu)
                        else:
                            nc.scalar.activation(
                                out=h1[:, :hw, :],
                                in_=ps2[:, :W_], func=AF.Relu)
                        xo = wpool.tile([128, 4, 128], f32, tag="xo")
                        r0 = (t0 + h * 4) * 128
                        for j in range(hw):
                            nc.sync.dma_start(
                                out=xo[:, j, :],
                                in_=xown_d[r0 + j * 128:r0 + (j + 1) * 128, :])

                        def layer_norm(dst_t, src_t, gb_off):
                            # per-tile LN over the feature (free) dim
                            s1 = stpool.tile([128, 4], f32, tag="s1")
                            nmu = stpool.tile([128, 4], f32, tag="nmu")
                            ss = stpool.tile([128, 4], f32, tag="ss")
                            sq = wpool.tile([128, 4, 128], f32, tag="sq")
                            std = stpool.tile([128, 4], f32, tag="std")
                            rstd = stpool.tile([128, 4], f32, tag="rstd")
                            nc.vector.tensor_reduce(
                                out=s1[:, :hw], in_=src_t[:, :hw, :],
                                axis=mybir.AxisListType.X, op=OP.add)
                            nc.vector.tensor_scalar_mul(
                                nmu[:, :hw], s1[:, :hw], -1.0 / C)
                            for j in range(hw):
                                nc.scalar.activation(
                                    out=sq[:, j, :], in_=src_t[:, j, :],
                                    func=AF.Square, bias=nmu[:, j:j + 1],
                                    accum_out=ss[:, j:j + 1])
                            nc.scalar.activation(
                                out=std[:, :hw], in_=ss[:, :hw],
                                func=AF.Sqrt, bias=eps_s[:, 0:1], scale=1.0 / C)
                            nc.vector.reciprocal(rstd[:, :hw], std[:, :hw])
                            for j in range(hw):
                                nc.vector.tensor_scalar(
                                    out=dst_t[:, j, :], in0=src_t[:, j, :],
                                    scalar1=nmu[:, j:j + 1],
                                    scalar2=rstd[:, j:j + 1],
                                    op0=OP.add, op1=OP.mult)
                            if gb_off is not None:
                                for j in range(hw):
                                    nc.vector.tensor_tensor(
                                        out=dst_t[:, j, :], in0=dst_t[:, j, :],
                                        in1=cvec_s[:, gb_off:gb_off + C],
                                        op=OP.mult)
                                    nc.vector.tensor_tensor(
                                        out=dst_t[:, j, :], in0=dst_t[:, j, :],
                                        in1=cvec_s[:, gb_off + C:gb_off + 2 * C],
                                        op=OP.add)

                        y1 = wpool.tile([128, 4, 128], f32, tag="y1")
                        layer_norm(y1, h1, C if apply_g1b1 else None)
                        h2 = wpool.tile([128, 4, 128], f32, tag="h2")
                        nc.vector.tensor_tensor(
                            out=h2[:, :hw, :], in0=y1[:, :hw, :],
                            in1=xo[:, :hw, :], op=OP.add)
                        ot = wpool.tile([128, 4, 128], f32, tag="ot")
                        layer_norm(ot, h2, None)
                        if apply_g2b2:
                            # gamma2/beta2 live at cvec offset C (g1b1 unused then)
                            pass
                        for j in range(hw):
                            nc.sync.dma_start(
                                out=out_d[r0 + j * 128:r0 + (j + 1) * 128, :],
                                in_=ot[:, j, :])
                assert q == nchunk
            if dummy_d is not None:
                nc.sync.dma_start(out=dummy_d[:], in_=eps_s[:])
    nc.compile()
    return nc


def _prep(cfg, x, edge_index, W, b, gamma1, beta1, gamma2, beta2):
    import ml_dtypes

    N, C, NCORES = cfg["N"], cfg["C"], cfg["NCORES"]
    npc, ntile, npad, nb, ngrp = _derived(cfg)
    src = np.asarray(edge_index[0], dtype=np.int64)
    dst = np.asarray(edge_index[1], dtype=np.int64)
    x = np.asarray(x, dtype=np.float32)
    W = np.asarray(W, dtype=np.float32)

    deg = (np.bincount(dst, minlength=N) + 1).astype(np.float32)
    dinv = (1.0 / np.sqrt(deg)).astype(np.float32)
    norm = (dinv[src] * dinv[dst]).astype(np.float32)

    sched, cores = _plan(cfg, src, dst, norm, dinv)

    gdt_np = np.float32 if cfg.get("F32TAB") else ml_dtypes.bfloat16
    xtab = np.ascontiguousarray(x.astype(gdt_np))
    wt = np.ascontiguousarray(W.T).astype(np.float32)
    iota = np.ascontiguousarray(np.broadcast_to(
        np.arange(128, dtype=np.float32), (128, 128)).astype(gdt_np))
    cvec = np.zeros((128, 3 * C), dtype=np.float32)
    cvec[:, 0:C] = b
    cvec[:, C:2 * C] = gamma1
    cvec[:, 2 * C:3 * C] = beta1
    # (gamma2/beta2 identity assumed; asserted by caller flags)

    sdt_np = gdt_np if cfg.get("SBATCH") else np.float32
    in_maps = []
    for c in range(NCORES):
        xo = np.zeros((npad, C), dtype=np.float32)
        xo[:npc] = x[c * npc:(c + 1) * npc]
        in_maps.append(dict(
            xtab=xtab, xown=xo, wt=wt, iota=iota,
            idx16=cores[c]["idx"],
            normT=np.ascontiguousarray(cores[c]["nrm"].astype(sdt_np)),
            dstlocT=np.ascontiguousarray(cores[c]["dlo"].astype(sdt_np)),
            cvec=cvec))
    return sched, in_maps


def _run(cfg, sched, in_maps, apply_bias, apply_g1b1, apply_g2b2, **kw):
    import time

    from concourse.bass_utils import run_bass_kernel_spmd

    t0 = time.time()
    nc = _build_nc(cfg, sched, apply_bias, apply_g1b1, apply_g2b2)
    print(f"[kernel] build+tile-schedule: {time.time() - t0:.1f}s",
          flush=True)
    t0 = time.time()
    res = run_bass_kernel_spmd(
        nc, in_maps, list(range(cfg["NCORES"])), **kw)
    print(f"[kernel] compile+run: {time.time() - t0:.1f}s", flush=True)
    return nc, res


def kernel(x, edge_index, W, b, gamma1, beta1, gamma2, beta2,
           _profile_out=None, _cfg_over=None):
    cfg = _cfg_full()
    if _cfg_over:
        cfg.update(_cfg_over)
    N, C = cfg["N"], cfg["C"]
    npc, ntile, npad, nb, ngrp = _derived(cfg)
    apply_bias = bool(np.any(np.asarray(b)))
    apply_g1b1 = not (np.all(np.asarray(gamma1) == 1)
                      and not np.any(np.asarray(beta1)))
    apply_g2b2 = not (np.all(np.asarray(gamma2) == 1)
                      and not np.any(np.asarray(beta2)))
    assert not apply_g2b2, "general gamma2/beta2 not wired"
    sched, in_maps = _prep(cfg, x, edge_index, W, b,
                           gamma1, beta1, gamma2, beta2)
    kw = {}
    if _profile_out is not None:
        kw = dict(trace=True, tmpdir=_profile_out)
    nc, res = _run(cfg, sched, in_maps, apply_bias, apply_g1b1, apply_g2b2,
                   **kw)
    outs = [res.results[c]["out"][:npc] for c in range(cfg["NCORES"])]
    full = np.concatenate(outs, axis=0).astype(np.float32)
    if _profile_out is not None:
        return full, res
    return full

